# revision 15
# baseline (speedup 1.0000x reference)
"""Trainium2 Bass kernel for nn_Attention_57080115364834.

Reference computation (B=4, C=512, H=W=64, N=H*W=4096 tokens):
    t = x.reshape(b, c, n).swapaxes(1, 2)          # (b, n, c)
    q, k, v = t@Wq.T+bq, t@Wk.T+bk, t@Wv.T+bv
    attn = softmax(q @ k.T / sqrt(c))              # (b, n, n)
    out = (attn @ v) @ Wo.T + bo                   # (b, n, c)
    return out.reshape(b, c, h, w)                 # raw view, no permute

Sharding: 8 cores = 4 batches x 2 query-halves. Each core holds the full
x[b] (C x N == t.T, the natural Trainium layout) so it computes its
batch's full K^T (c,n) and VW (n,c) locally, plus Q^T for its half.

Host-side algebra folds both post-attention linear steps away:
  - softmax rows sum to 1  =>  v bias becomes output bias bo' = Wo@bv+bo,
    applied on the host after gathering (a per-channel constant add).
  - (attn@v)@Wo.T == attn@(t@(Wo@Wv).T), so with Wvo = Wo@Wv precomputed
    on host the VW projection directly produces final-channel values.

Per-core dataflow (matmuls bf16, f32 PSUM; normalization f32/f32r):
  kT[c,m]   = Wk @ tC + bk    VW[m,c] = tC.T @ WvoT     (phase 1)
  qT[c,n]   = Wq @ tCq + bq  per 512-token chunk (chunk 0 hoisted into
              phase 1 so the PSUM pool handover never idles the PE)
  ST[m,n]   = kT.T @ qT ; P = exp(ST/sqrt(c)) on ScalarE (no max-sub)
  acc      += P (DVE, f32r, for the rowsums)
  OT[n,c]  += P-chunk.T @ VW          (PSUM-accum over m-tiles; output is
              token-major, so rowsums live one-per-partition)
  otsb      = OT   (PSUM->SBUF split DVE/ACT; frees the 4 OT banks so the
              next chunk's attnv matmuls never wait)
  rs[tl]    = acc-chunk.T @ ones      (4 tiny N=1 matmuls -> [128,4])
  out[n,c]  = otsb / rs  via GPSIMD normalize_recip (per-partition denom)
              -> DMA, token-major, no transpose needed anywhere.

The old channel-major design needed reciprocal+broadcast of a [1,512]
row: a single-lane 3.3us DVE reciprocal plus a K=1 broadcast matmul sat
in the serial chain of every chunk and fully exposed at kernel end.
Token-major output turns that into per-partition work on the otherwise
idle GPSIMD engine.

Engine queues are strict FIFO, so chunk nb's normalization ops are
spread across chunk nb+1 (rowsums after qproj, normalize+DMA at mt==2)
so no FIFO ever backs up behind a dependency chain.

A short warm-up burst of zero matmuls runs during the initial DMA
lead-in so the PE_HAM clock gate is already 8/8 when real work arrives.
"""

import sys

for _p in ("/opt/trn_rl_repo", "/root/.axon_site/_ro/trn_rl_repo"):
    if _p not in sys.path:
        sys.path.append(_p)

import numpy as np
import ml_dtypes

import concourse.bacc as bacc
import concourse.mybir as mybir
import concourse.tile as tile
from concourse.bass_utils import run_bass_kernel_spmd

DT = mybir.dt.float32
FR = mybir.dt.float32r
BF = mybir.dt.bfloat16
AFT = mybir.ActivationFunctionType

B, C, HW = 4, 512, 4096          # batch, channels, tokens per batch
NQ = HW // 2                     # q tokens per core (2048)
CK = C // 128                    # contraction chunks (4)
MT = HW // 128                   # key/value tiles (32)
NB = NQ // 512                   # q-chunks per core (4)
SCALE = 1.0 / float(np.sqrt(C))
N_CORES = 8
N_WARM = 20                      # HAM warm-up matmuls

_compiled = None
_ONES_BF = np.ones(128, dtype=ml_dtypes.bfloat16)


def _build():
    nc = bacc.Bacc("TRN2", target_bir_lowering=False)

    xt_e = nc.declare_dram_parameter("xt", [C, HW], BF, isOutput=False)
    xq_e = nc.declare_dram_parameter("xq", [C, NQ], BF, isOutput=False)
    wqt_e = nc.declare_dram_parameter("wqt", [C, C], BF, isOutput=False)
    wkt_e = nc.declare_dram_parameter("wkt", [C, C], BF, isOutput=False)
    wvot_e = nc.declare_dram_parameter("wvot", [C, C], BF, isOutput=False)
    bq_e = nc.declare_dram_parameter("bq", [C], DT, isOutput=False)
    bk_e = nc.declare_dram_parameter("bk", [C], DT, isOutput=False)
    ones_bf_e = nc.declare_dram_parameter("ones_bf", [128], BF, isOutput=False)
    out_e = nc.declare_dram_parameter("out", [NQ, C], DT, isOutput=True)

    with tile.TileContext(nc) as tc:
        # ---- HAM warm-up: zero matmuls with no DMA dependency keep the
        # PE busy through the initial DMA lead-in so the clock gate is at
        # 8/8 when real matmuls arrive. Pool closes -> PSUM bank reused.
        with (
            tc.tile_pool(name="warm", bufs=1) as warm_pool,
            tc.tile_pool(name="warmps", bufs=1, space="PSUM") as warm_ps,
        ):
            warm_sb = warm_pool.tile([128, 512], BF, tag="warm", name="warm_sb")
            nc.gpsimd.memset(warm_sb[:], 0.0)
            warm_ps_t = warm_ps.tile([128, 512], DT, tag="warmps", name="warm_ps")
            for i in range(N_WARM):
                nc.tensor.matmul(
                    warm_ps_t[:], warm_sb[:, 0:128], warm_sb[:],
                    start=(i == 0), stop=(i == N_WARM - 1),
                )

        with (
            tc.tile_pool(name="kt", bufs=1) as kt_pool,
            tc.tile_pool(name="vv", bufs=1) as vv_pool,
            tc.tile_pool(name="wq", bufs=1) as wq_pool,
            tc.tile_pool(name="consts", bufs=1) as c_pool,
            tc.tile_pool(name="xqp", bufs=2) as xq_pool,
            tc.tile_pool(name="qcp", bufs=2) as qc_pool,
        ):
            # ---- persistent tiles ----
            kt_sb = [kt_pool.tile([128, HW], BF, tag=f"k{i}", name=f"k{i}") for i in range(CK)]
            vw_sb = [vv_pool.tile([128, C], BF, tag=f"v{i}", name=f"v{i}") for i in range(MT)]
            wq_sb = [wq_pool.tile([128, C], BF, tag=f"wq{i}", name=f"wq{i}") for i in range(CK)]

            bq_t = c_pool.tile([128, CK], DT, tag="bq", name="bq_t")
            bk_t = c_pool.tile([128, CK], DT, tag="bk", name="bk_t")
            ones_col_b = c_pool.tile([128, 1], BF, tag="onescb", name="ones_col_b")

            def project_q(nb, ps_pool, ps_tag, ps_bufs):
                xqs = [xq_pool.tile([128, 512], BF, tag=f"xq{ci}", name=f"xq{ci}") for ci in range(CK)]
                for ci in range(CK):
                    nc.gpsimd.dma_start(
                        xqs[ci][:], xq_e[ci * 128:(ci + 1) * 128, nb * 512:(nb + 1) * 512]
                    )
                qcs = []
                for co in range(CK):
                    pq = ps_pool.tile([128, 512], DT, tag=ps_tag, name="pq", bufs=ps_bufs)
                    for ci in range(CK):
                        nc.tensor.matmul(
                            pq[:], wq_sb[ci][:, co * 128:(co + 1) * 128],
                            xqs[ci][:], start=(ci == 0), stop=(ci == CK - 1),
                        )
                    qc = qc_pool.tile([128, 512], BF, tag=f"qc{co}", name=f"qc{co}")
                    nc.scalar.activation(qc[:], pq[:], AFT.Identity, bias=bq_t[:, co:co + 1])
                    qcs.append(qc)
                return qcs

            # ---- phase 1: kT (c,m) and VW (m,c) projections ----
            with (
                tc.tile_pool(name="wkv", bufs=1) as wkv_pool,
                tc.tile_pool(name="tcc", bufs=3) as tcc_pool,
                tc.tile_pool(name="ps1", bufs=2, space="PSUM") as ps1,
            ):
                wk_sb = [wkv_pool.tile([128, C], BF, tag=f"wk{i}", name=f"wk{i}") for i in range(CK)]
                wv_sb = [wkv_pool.tile([128, C], BF, tag=f"wv{i}", name=f"wv{i}") for i in range(CK)]

                # lead-in-critical DMAs first, in consumption order
                tcs01 = [tcc_pool.tile([128, 512], BF, tag=f"tc{ci}", name=f"tc{ci}") for ci in range(CK)]
                for ci in range(CK):
                    nc.sync.dma_start(tcs01[ci][:], xt_e[ci * 128:(ci + 1) * 128, 0:512])
                for i in range(CK):
                    nc.sync.dma_start(wk_sb[i][:], wkt_e[i * 128:(i + 1) * 128, :])
                for i in range(CK):
                    nc.sync.dma_start(wv_sb[i][:], wvot_e[i * 128:(i + 1) * 128, :])
                tcs1 = [tcc_pool.tile([128, 512], BF, tag=f"tc{ci}", name=f"tc{ci}") for ci in range(CK)]
                for ci in range(CK):
                    nc.sync.dma_start(tcs1[ci][:], xt_e[ci * 128:(ci + 1) * 128, 512:1024])
                for t in range(CK):
                    nc.sync.dma_start(bk_t[:, t:t + 1], bk_e[t * 128:(t + 1) * 128])
                nc.sync.dma_start(ones_col_b[:, 0:1], ones_bf_e[:])
                for i in range(CK):
                    nc.sync.dma_start(wq_sb[i][:], wqt_e[i * 128:(i + 1) * 128, :])
                for t in range(CK):
                    nc.sync.dma_start(bq_t[:, t:t + 1], bq_e[t * 128:(t + 1) * 128])

                qcs0 = None
                for j in range(HW // 512):
                    if j == 0:
                        tcs = tcs01
                    elif j == 1:
                        tcs = tcs1
                    else:
                        tcs = [tcc_pool.tile([128, 512], BF, tag=f"tc{ci}", name=f"tc{ci}") for ci in range(CK)]
                        for ci in range(CK):
                            nc.gpsimd.dma_start(
                                tcs[ci][:], xt_e[ci * 128:(ci + 1) * 128, j * 512:(j + 1) * 512]
                            )
                    # kT token-chunk j, all four output-channel chunks
                    for co in range(CK):
                        pk = ps1.tile([128, 512], DT, tag="pk", name="pk")
                        for ci in range(CK):
                            nc.tensor.matmul(
                                pk[:], wk_sb[ci][:, co * 128:(co + 1) * 128],
                                tcs[ci][:], start=(ci == 0), stop=(ci == CK - 1),
                            )
                        nc.scalar.activation(
                            kt_sb[co][:, j * 512:(j + 1) * 512], pk[:], AFT.Identity,
                            bias=bk_t[:, co:co + 1],
                        )
                    # VW m-tiles 4j..4j+3 (no bias: folded into bo'),
                    # evacuation split DVE/ACT to balance engine backlogs.
                    for ml in range(4):
                        pv = ps1.tile([128, 512], DT, tag="pv", name="pv")
                        for ci in range(CK):
                            nc.tensor.matmul(
                                pv[:], tcs[ci][:, ml * 128:(ml + 1) * 128],
                                wv_sb[ci][:], start=(ci == 0), stop=(ci == CK - 1),
                            )
                        if ml % 2 == 0:
                            nc.vector.tensor_copy(vw_sb[4 * j + ml][:], pv[:])
                        else:
                            nc.scalar.activation(vw_sb[4 * j + ml][:], pv[:], AFT.Copy)
                    # chunk-0 q projection rides inside phase 1 so phase 2
                    # can open with score matmuls immediately
                    if j == 4:
                        qcs0 = project_q(0, ps1, "pq", 1)

            # ---- phase 2: attention per 512-token q-chunk ----
            with (
                tc.tile_pool(name="pexp", bufs=8) as pe_pool,
                tc.tile_pool(name="accp", bufs=2) as acc_pool,
                tc.tile_pool(name="otsbp", bufs=2) as otsb_pool,
                tc.tile_pool(name="outp", bufs=3) as out_pool,
                tc.tile_pool(name="smallp", bufs=2) as small_pool,
                tc.tile_pool(name="ps2", bufs=4, space="PSUM") as ps2,
                tc.tile_pool(name="psot", bufs=1, space="PSUM") as psot,
            ):
                def tail_rowsums(acc_bf):
                    # transposed rowsums: [128 tokens, 1] per 128-token group,
                    # so the denominators land one-per-partition (fp32r can't
                    # be a wide stationary operand, hence the bf16 acc copy;
                    # the 128-way sum averages the rounding noise to ~4e-4)
                    rs_ps = ps2.tile([128, CK], DT, tag="st", name="rs_ps")
                    for tl in range(CK):
                        nc.tensor.matmul(
                            rs_ps[:, tl:tl + 1], acc_bf[:, tl * 128:(tl + 1) * 128],
                            ones_col_b[:, 0:1],
                            start=(tl == 0), stop=(tl == CK - 1),
                            skip_group_check=(tl > 0),
                        )
                    rs_sb = small_pool.tile([128, CK], DT, tag="rssb", name="rs_sb")
                    nc.vector.tensor_copy(rs_sb[:], rs_ps[:])
                    return rs_sb

                def tail_norm_out(tnb, tl, otsb_t, rs_sb):
                    fin = out_pool.tile([128, C], DT, tag="fin", name="fin", bufs=5)
                    nc.gpsimd.normalize_recip(fin[:], otsb_t[:], rs_sb[:, tl:tl + 1])
                    nc.gpsimd.dma_start(
                        out_e[tnb * 512 + tl * 128:tnb * 512 + (tl + 1) * 128, :], fin[:]
                    )

                prev = None
                for nb in range(NB):
                    qcs = qcs0 if nb == 0 else project_q(nb, ps2, "st", None)
                    if prev is not None:
                        prev = (*prev, tail_rowsums(prev[1]))

                    # bf16 accumulator: 2x DVE rate, valid as a matmul
                    # stationary operand, and the rounding noise averages to
                    # ~4e-4 across the 128-way rowsum
                    acc = acc_pool.tile([128, 512], BF, tag="acc", name="acc")
                    ots = [psot.tile([128, C], DT, tag=f"ot{tl}", name=f"ot{tl}") for tl in range(CK)]
                    for mt in range(MT):
                        st = ps2.tile([128, 512], DT, tag="st", name="st")
                        for ci in range(CK):
                            nc.tensor.matmul(
                                st[:], kt_sb[ci][:, mt * 128:(mt + 1) * 128],
                                qcs[ci][:], start=(ci == 0), stop=(ci == CK - 1),
                            )
                        pexp = pe_pool.tile([128, 512], BF, tag="pe", name="pexp")
                        nc.scalar.activation(pexp[:], st[:], AFT.Exp, scale=SCALE)
                        if mt == 0:
                            nc.vector.tensor_copy(acc[:], pexp[:])
                        else:
                            nc.vector.tensor_add(acc[:], acc[:], pexp[:])
                        for tl in range(CK):
                            nc.tensor.matmul(
                                ots[tl][:], pexp[:, tl * 128:(tl + 1) * 128],
                                vw_sb[mt][:],
                                start=(mt == 0), stop=(mt == MT - 1), skip_group_check=True,
                            )
                        if mt == 2 and prev is not None:
                            tnb, _, otsb, rs_sb = prev
                            for tl in range(CK):
                                tail_norm_out(tnb, tl, otsb[tl], rs_sb)

                    # evacuate unnormalized accumulators (DVE/ACT split) ->
                    # frees the 4 OT PSUM banks for the next chunk at once
                    otsb = []
                    for tl in range(CK):
                        ob = otsb_pool.tile([128, C], DT, tag=f"ob{tl}", name=f"ob{tl}")
                        if tl % 2 == 0:
                            nc.vector.tensor_copy(ob[:], ots[tl][:])
                        else:
                            nc.scalar.activation(ob[:], ots[tl][:], AFT.Copy)
                        otsb.append(ob)
                    prev = (nb, acc, otsb)

                # last chunk: emit its whole tail immediately
                tnb, acc, otsb = prev
                rs_sb = tail_rowsums(acc)
                for tl in range(CK):
                    tail_norm_out(tnb, tl, otsb[tl], rs_sb)

    nc.compile()
    return nc


def _get_compiled():
    global _compiled
    if _compiled is None:
        _compiled = _build()
    return _compiled


def kernel(**inputs):
    x = np.ascontiguousarray(np.asarray(inputs["x"], dtype=np.float32))
    wq = np.asarray(inputs["Wq"], dtype=np.float32)
    wk = np.asarray(inputs["Wk"], dtype=np.float32)
    wv = np.asarray(inputs["Wv"], dtype=np.float32)
    wo = np.asarray(inputs["Wo"], dtype=np.float32)
    bq = np.ascontiguousarray(np.asarray(inputs["bq"], dtype=np.float32))
    bk = np.ascontiguousarray(np.asarray(inputs["bk"], dtype=np.float32))
    bv = np.asarray(inputs["bv"], dtype=np.float32)
    bo = np.asarray(inputs["bo"], dtype=np.float32)

    wqt = np.ascontiguousarray(wq.T.astype(ml_dtypes.bfloat16))
    wkt = np.ascontiguousarray(wk.T.astype(ml_dtypes.bfloat16))
    wvot = np.ascontiguousarray((wo @ wv).T.astype(ml_dtypes.bfloat16))
    bop = wo @ bv + bo

    xb = x.reshape(B, C, HW).astype(ml_dtypes.bfloat16)
    in_maps = []
    for core in range(N_CORES):
        bi, h = core // 2, core % 2
        in_maps.append({
            "xt": np.ascontiguousarray(xb[bi]),
            "xq": np.ascontiguousarray(xb[bi][:, h * NQ:(h + 1) * NQ]),
            "wqt": wqt, "wkt": wkt, "wvot": wvot,
            "bq": bq, "bk": bk, "ones_bf": _ONES_BF,
        })

    nc = _get_compiled()
    res = run_bass_kernel_spmd(nc, in_maps, core_ids=list(range(N_CORES)))

    out = np.empty((B, HW, C), dtype=np.float32)
    for core in range(N_CORES):
        bi, h = core // 2, core % 2
        out[bi, h * NQ:(h + 1) * NQ, :] = res.results[core]["out"]
    out += bop  # bo' = Wo@bv + bo, exact because softmax rows sum to 1
    return out.reshape(B, C, 64, 64)


# revision 22
# speedup vs baseline: 1.3378x; 1.3378x over previous
"""Trainium2 Bass kernel for nn_Attention_57080115364834.

Reference computation (B=4, C=512, H=W=64, N=H*W=4096 tokens):
    t = x.reshape(b, c, n).swapaxes(1, 2)          # (b, n, c)
    q, k, v = t@Wq.T+bq, t@Wk.T+bk, t@Wv.T+bv
    attn = softmax(q @ k.T / sqrt(c))              # (b, n, n)
    out = (attn @ v) @ Wo.T + bo                   # (b, n, c)
    return out.reshape(b, c, h, w)                 # raw view, no permute

Sharding: 8 cores = 4 batches x 2 query-halves. Each core holds the full
x[b] (C x N == t.T, the natural Trainium layout) so it computes its
batch's full K^T (c,n) and VW (n,c) locally, plus Q^T for its half.

Host-side algebra folds both post-attention linear steps away:
  - softmax rows sum to 1  =>  v bias becomes output bias bo' = Wo@bv+bo,
    applied on the host after gathering (a per-channel constant add).
  - (attn@v)@Wo.T == attn@(t@(Wo@Wv).T), so with Wvo = Wo@Wv precomputed
    on host the VW projection directly produces final-channel values.

Per-core dataflow (matmuls bf16, f32 PSUM; normalization f32/f32r):
  kT[c,m]   = Wk @ tC + bk    VW[m,c] = tC.T @ WvoT     (phase 1)
  qT[c,n]   = Wq @ tCq + bq  per 512-token chunk (chunk 0 hoisted into
              phase 1 so the PSUM pool handover never idles the PE)
  ST[m,n]   = kT.T @ qT ; P = exp(ST/sqrt(c)) on ScalarE (no max-sub)
  acc      += P (DVE, f32r, for the rowsums)
  OT[n,c]  += P-chunk.T @ VW          (PSUM-accum over m-tiles; output is
              token-major, so rowsums live one-per-partition)
  otsb      = OT   (PSUM->SBUF split DVE/ACT; frees the 4 OT banks so the
              next chunk's attnv matmuls never wait)
  rs[tl]    = acc-chunk.T @ ones      (4 tiny N=1 matmuls -> [128,4])
  out[n,c]  = otsb / rs  via GPSIMD normalize_recip (per-partition denom)
              -> DMA, token-major, no transpose needed anywhere.

The old channel-major design needed reciprocal+broadcast of a [1,512]
row: a single-lane 3.3us DVE reciprocal plus a K=1 broadcast matmul sat
in the serial chain of every chunk and fully exposed at kernel end.
Token-major output turns that into per-partition work on the otherwise
idle GPSIMD engine.

Engine queues are strict FIFO, so chunk nb's normalization ops are
spread across chunk nb+1 (rowsums after qproj, normalize+DMA at mt==2)
so no FIFO ever backs up behind a dependency chain.

A short warm-up burst of zero matmuls runs during the initial DMA
lead-in so the PE_HAM clock gate is already 8/8 when real work arrives.
"""

import sys

for _p in ("/opt/trn_rl_repo", "/root/.axon_site/_ro/trn_rl_repo"):
    if _p not in sys.path:
        sys.path.append(_p)

import numpy as np
import ml_dtypes

import concourse.bacc as bacc
import concourse.mybir as mybir
import concourse.tile as tile
from concourse.bass_utils import run_bass_kernel_spmd

DT = mybir.dt.float32
BF = mybir.dt.bfloat16
F8 = mybir.dt.float8e4
AFT = mybir.ActivationFunctionType
DR = mybir.MatmulPerfMode.DoubleRow

B, C, HW = 4, 512, 4096          # batch, channels, tokens per batch
NQ = HW // 2                     # q tokens per core (2048)
CK = C // 128                    # contraction chunks (4)
MT = HW // 128                   # key/value tiles (32)
NB = NQ // 512                   # q-chunks per core (4)
SCALE = 1.0 / float(np.sqrt(C))
N_CORES = 8
N_WARM = 20                      # HAM warm-up matmuls
MT8 = 24                         # m-tiles 0..MT8-1 scored in fp8 DoubleRow
                                 # (keys are order-invariant under softmax;
                                 # fp8 on 3/4 of them measures ~1.4e-2 total
                                 # rel err vs the 2e-2 gate, and doubles the
                                 # score-matmul rate on those tiles)
J8 = MT8 // 4                    # 512-token kT slices stored fp8 (6)

_compiled = None
_ONES_BF = np.ones(128, dtype=ml_dtypes.bfloat16)


def _build():
    nc = bacc.Bacc("TRN2", target_bir_lowering=False)

    xt_e = nc.declare_dram_parameter("xt", [C, HW], BF, isOutput=False)
    xq_e = nc.declare_dram_parameter("xq", [C, NQ], BF, isOutput=False)
    wqt_e = nc.declare_dram_parameter("wqt", [C, C], BF, isOutput=False)
    wkt_e = nc.declare_dram_parameter("wkt", [C, C], BF, isOutput=False)
    wvot_e = nc.declare_dram_parameter("wvot", [C, C], BF, isOutput=False)
    bq_e = nc.declare_dram_parameter("bq", [C], DT, isOutput=False)
    bk_e = nc.declare_dram_parameter("bk", [C], DT, isOutput=False)
    ones_bf_e = nc.declare_dram_parameter("ones_bf", [128], BF, isOutput=False)
    out_e = nc.declare_dram_parameter("out", [NQ, C], DT, isOutput=True)

    with tile.TileContext(nc) as tc:
        # ---- HAM warm-up: zero matmuls with no DMA dependency keep the
        # PE busy through the initial DMA lead-in so the clock gate is at
        # 8/8 when real matmuls arrive. Pool closes -> PSUM bank reused.
        with (
            tc.tile_pool(name="warm", bufs=1) as warm_pool,
            tc.tile_pool(name="warmps", bufs=1, space="PSUM") as warm_ps,
        ):
            warm_sb = warm_pool.tile([128, 512], BF, tag="warm", name="warm_sb")
            nc.gpsimd.memset(warm_sb[:], 0.0)
            warm_ps_t = warm_ps.tile([128, 512], DT, tag="warmps", name="warm_ps")
            for i in range(N_WARM):
                nc.tensor.matmul(
                    warm_ps_t[:], warm_sb[:, 0:128], warm_sb[:],
                    start=(i == 0), stop=(i == N_WARM - 1),
                )

        with (
            tc.tile_pool(name="kt", bufs=1) as kt_pool,
            tc.tile_pool(name="vv", bufs=1) as vv_pool,
            tc.tile_pool(name="wq", bufs=1) as wq_pool,
            tc.tile_pool(name="consts", bufs=1) as c_pool,
            tc.tile_pool(name="xqp", bufs=2) as xq_pool,
            tc.tile_pool(name="qcp", bufs=2) as qc_pool,
        ):
            # ---- persistent tiles ----
            # kT split by score precision: m-tiles < MT8 live fp8 in the
            # DoubleRow-paired layout [128, 2(channel-group), m], the rest bf16
            kt8 = [kt_pool.tile([128, 2, J8 * 512], F8, tag=f"k8{g}", name=f"k8{g}") for g in range(2)]
            kt_bf = [kt_pool.tile([128, (MT - MT8) * 128], BF, tag=f"k{i}", name=f"k{i}") for i in range(CK)]
            vw_sb = [vv_pool.tile([128, C], BF, tag=f"v{i}", name=f"v{i}") for i in range(MT)]
            wq_sb = [wq_pool.tile([128, C], BF, tag=f"wq{i}", name=f"wq{i}") for i in range(CK)]

            bq_t = c_pool.tile([128, CK], DT, tag="bq", name="bq_t")
            bk_t = c_pool.tile([128, CK], DT, tag="bk", name="bk_t")
            ones_col_b = c_pool.tile([128, 1], BF, tag="onescb", name="ones_col_b")

            def project_q(nb, ps_pool, ps_tag, ps_bufs):
                xqs = [xq_pool.tile([128, 512], BF, tag=f"xq{ci}", name=f"xq{ci}") for ci in range(CK)]
                for ci in range(CK):
                    nc.gpsimd.dma_start(
                        xqs[ci][:], xq_e[ci * 128:(ci + 1) * 128, nb * 512:(nb + 1) * 512]
                    )
                qcs = []
                qc8 = [qc_pool.tile([128, 2, 512], F8, tag=f"qc8{g}", name=f"qc8{g}") for g in range(2)]
                for co in range(CK):
                    pq = ps_pool.tile([128, 512], DT, tag=ps_tag, name="pq", bufs=ps_bufs)
                    for ci in range(CK):
                        nc.tensor.matmul(
                            pq[:], wq_sb[ci][:, co * 128:(co + 1) * 128],
                            xqs[ci][:], start=(ci == 0), stop=(ci == CK - 1),
                        )
                    qc = qc_pool.tile([128, 512], BF, tag=f"qc{co}", name=f"qc{co}")
                    nc.scalar.activation(qc[:], pq[:], AFT.Identity, bias=bq_t[:, co:co + 1])
                    nc.scalar.activation(qc8[co // 2][:, co % 2, :], pq[:], AFT.Identity,
                                         bias=bq_t[:, co:co + 1])
                    qcs.append(qc)
                return qcs, qc8

            # ---- phase 1: kT (c,m) and VW (m,c) projections ----
            with (
                tc.tile_pool(name="wkv", bufs=1) as wkv_pool,
                tc.tile_pool(name="tcc", bufs=3) as tcc_pool,
                tc.tile_pool(name="ps1", bufs=2, space="PSUM") as ps1,
            ):
                wk_sb = [wkv_pool.tile([128, C], BF, tag=f"wk{i}", name=f"wk{i}") for i in range(CK)]
                wv_sb = [wkv_pool.tile([128, C], BF, tag=f"wv{i}", name=f"wv{i}") for i in range(CK)]

                # lead-in-critical DMAs first, in consumption order
                tcs01 = [tcc_pool.tile([128, 512], BF, tag=f"tc{ci}", name=f"tc{ci}") for ci in range(CK)]
                for ci in range(CK):
                    nc.sync.dma_start(tcs01[ci][:], xt_e[ci * 128:(ci + 1) * 128, 0:512])
                for i in range(CK):
                    nc.sync.dma_start(wk_sb[i][:], wkt_e[i * 128:(i + 1) * 128, :])
                for i in range(CK):
                    nc.sync.dma_start(wv_sb[i][:], wvot_e[i * 128:(i + 1) * 128, :])
                tcs1 = [tcc_pool.tile([128, 512], BF, tag=f"tc{ci}", name=f"tc{ci}") for ci in range(CK)]
                for ci in range(CK):
                    nc.sync.dma_start(tcs1[ci][:], xt_e[ci * 128:(ci + 1) * 128, 512:1024])
                for t in range(CK):
                    nc.sync.dma_start(bk_t[:, t:t + 1], bk_e[t * 128:(t + 1) * 128])
                nc.sync.dma_start(ones_col_b[:, 0:1], ones_bf_e[:])
                for i in range(CK):
                    nc.sync.dma_start(wq_sb[i][:], wqt_e[i * 128:(i + 1) * 128, :])
                for t in range(CK):
                    nc.sync.dma_start(bq_t[:, t:t + 1], bq_e[t * 128:(t + 1) * 128])

                qcs0 = None
                for j in range(HW // 512):
                    if j == 0:
                        tcs = tcs01
                    elif j == 1:
                        tcs = tcs1
                    else:
                        tcs = [tcc_pool.tile([128, 512], BF, tag=f"tc{ci}", name=f"tc{ci}") for ci in range(CK)]
                        for ci in range(CK):
                            nc.gpsimd.dma_start(
                                tcs[ci][:], xt_e[ci * 128:(ci + 1) * 128, j * 512:(j + 1) * 512]
                            )
                    # kT token-chunk j, all four output-channel chunks
                    for co in range(CK):
                        pk = ps1.tile([128, 512], DT, tag="pk", name="pk")
                        for ci in range(CK):
                            nc.tensor.matmul(
                                pk[:], wk_sb[ci][:, co * 128:(co + 1) * 128],
                                tcs[ci][:], start=(ci == 0), stop=(ci == CK - 1),
                            )
                        if j < J8:
                            dst = kt8[co // 2][:, co % 2, j * 512:(j + 1) * 512]
                        else:
                            dst = kt_bf[co][:, (j - J8) * 512:(j - J8 + 1) * 512]
                        nc.scalar.activation(dst, pk[:], AFT.Identity,
                                             bias=bk_t[:, co:co + 1])
                    # VW m-tiles 4j..4j+3 (no bias: folded into bo'),
                    # evacuation split DVE/ACT to balance engine backlogs.
                    for ml in range(4):
                        pv = ps1.tile([128, 512], DT, tag="pv", name="pv")
                        for ci in range(CK):
                            nc.tensor.matmul(
                                pv[:], tcs[ci][:, ml * 128:(ml + 1) * 128],
                                wv_sb[ci][:], start=(ci == 0), stop=(ci == CK - 1),
                            )
                        if ml % 2 == 0:
                            nc.vector.tensor_copy(vw_sb[4 * j + ml][:], pv[:])
                        else:
                            nc.scalar.activation(vw_sb[4 * j + ml][:], pv[:], AFT.Copy)
                    # chunk-0 q projection rides inside phase 1 so phase 2
                    # can open with score matmuls immediately
                    if j == 4:
                        qcs0, qc80 = project_q(0, ps1, "pq", 1)

            # ---- phase 2: attention per 512-token q-chunk ----
            with (
                tc.tile_pool(name="pexp", bufs=8) as pe_pool,
                tc.tile_pool(name="accp", bufs=2) as acc_pool,
                tc.tile_pool(name="otsbp", bufs=2) as otsb_pool,
                tc.tile_pool(name="outp", bufs=3) as out_pool,
                tc.tile_pool(name="smallp", bufs=2) as small_pool,
                tc.tile_pool(name="ps2", bufs=4, space="PSUM") as ps2,
                tc.tile_pool(name="psot", bufs=1, space="PSUM") as psot,
            ):
                def tail_rowsums(acc_bf):
                    # transposed rowsums: [128 tokens, 1] per 128-token group,
                    # so the denominators land one-per-partition (fp32r can't
                    # be a wide stationary operand, hence the bf16 acc copy;
                    # the 128-way sum averages the rounding noise to ~4e-4)
                    rs_ps = ps2.tile([128, CK], DT, tag="st", name="rs_ps")
                    for tl in range(CK):
                        nc.tensor.matmul(
                            rs_ps[:, tl:tl + 1], acc_bf[:, tl * 128:(tl + 1) * 128],
                            ones_col_b[:, 0:1],
                            start=(tl == 0), stop=(tl == CK - 1),
                            skip_group_check=(tl > 0),
                        )
                    rs_sb = small_pool.tile([128, CK], DT, tag="rssb", name="rs_sb")
                    nc.vector.tensor_copy(rs_sb[:], rs_ps[:])
                    return rs_sb

                def tail_norm_out(tnb, tl, otsb_t, rs_sb):
                    fin = out_pool.tile([128, C], DT, tag="fin", name="fin", bufs=5)
                    nc.gpsimd.normalize_recip(fin[:], otsb_t[:], rs_sb[:, tl:tl + 1])
                    nc.gpsimd.dma_start(
                        out_e[tnb * 512 + tl * 128:tnb * 512 + (tl + 1) * 128, :], fin[:]
                    )

                prev = None
                for nb in range(NB):
                    qcs, qc8 = (qcs0, qc80) if nb == 0 else project_q(nb, ps2, "st", None)
                    if prev is not None:
                        prev = (*prev, tail_rowsums(prev[1]))

                    # bf16 accumulator: 2x DVE rate, valid as a matmul
                    # stationary operand, and the rounding noise averages to
                    # ~4e-4 across the 128-way rowsum
                    acc = acc_pool.tile([128, 512], BF, tag="acc", name="acc")
                    ots = [psot.tile([128, C], DT, tag=f"ot{tl}", name=f"ot{tl}") for tl in range(CK)]
                    for mt in range(MT):
                        st = ps2.tile([128, 512], DT, tag="st", name="st")
                        if mt < MT8:
                            for g in range(2):
                                nc.tensor.matmul(
                                    st[:], kt8[g][:, :, mt * 128:(mt + 1) * 128],
                                    qc8[g][:, :, :], start=(g == 0), stop=(g == 1),
                                    perf_mode=DR,
                                )
                        else:
                            for ci in range(CK):
                                nc.tensor.matmul(
                                    st[:], kt_bf[ci][:, (mt - MT8) * 128:(mt - MT8 + 1) * 128],
                                    qcs[ci][:], start=(ci == 0), stop=(ci == CK - 1),
                                )
                        pexp = pe_pool.tile([128, 512], BF, tag="pe", name="pexp")
                        nc.scalar.activation(pexp[:], st[:], AFT.Exp, scale=SCALE)
                        if mt == 0:
                            nc.vector.tensor_copy(acc[:], pexp[:])
                        else:
                            nc.vector.tensor_add(acc[:], acc[:], pexp[:])
                        for tl in range(CK):
                            nc.tensor.matmul(
                                ots[tl][:], pexp[:, tl * 128:(tl + 1) * 128],
                                vw_sb[mt][:],
                                start=(mt == 0), stop=(mt == MT - 1), skip_group_check=True,
                            )
                        if mt == 2 and prev is not None:
                            tnb, _, otsb, rs_sb = prev
                            for tl in range(CK):
                                tail_norm_out(tnb, tl, otsb[tl], rs_sb)

                    # evacuate unnormalized accumulators (DVE/ACT split) ->
                    # frees the 4 OT PSUM banks for the next chunk at once
                    otsb = []
                    for tl in range(CK):
                        ob = otsb_pool.tile([128, C], DT, tag=f"ob{tl}", name=f"ob{tl}")
                        if tl % 2 == 0:
                            nc.vector.tensor_copy(ob[:], ots[tl][:])
                        else:
                            nc.scalar.activation(ob[:], ots[tl][:], AFT.Copy)
                        otsb.append(ob)
                    prev = (nb, acc, otsb)

                # last chunk: emit its whole tail immediately
                tnb, acc, otsb = prev
                rs_sb = tail_rowsums(acc)
                for tl in range(CK):
                    tail_norm_out(tnb, tl, otsb[tl], rs_sb)

    nc.compile()
    return nc


def _get_compiled():
    global _compiled
    if _compiled is None:
        _compiled = _build()
    return _compiled


def kernel(**inputs):
    x = np.ascontiguousarray(np.asarray(inputs["x"], dtype=np.float32))
    wq = np.asarray(inputs["Wq"], dtype=np.float32)
    wk = np.asarray(inputs["Wk"], dtype=np.float32)
    wv = np.asarray(inputs["Wv"], dtype=np.float32)
    wo = np.asarray(inputs["Wo"], dtype=np.float32)
    bq = np.ascontiguousarray(np.asarray(inputs["bq"], dtype=np.float32))
    bk = np.ascontiguousarray(np.asarray(inputs["bk"], dtype=np.float32))
    bv = np.asarray(inputs["bv"], dtype=np.float32)
    bo = np.asarray(inputs["bo"], dtype=np.float32)

    wqt = np.ascontiguousarray(wq.T.astype(ml_dtypes.bfloat16))
    wkt = np.ascontiguousarray(wk.T.astype(ml_dtypes.bfloat16))
    wvot = np.ascontiguousarray((wo @ wv).T.astype(ml_dtypes.bfloat16))
    bop = wo @ bv + bo

    xb = x.reshape(B, C, HW).astype(ml_dtypes.bfloat16)
    in_maps = []
    for core in range(N_CORES):
        bi, h = core // 2, core % 2
        in_maps.append({
            "xt": np.ascontiguousarray(xb[bi]),
            "xq": np.ascontiguousarray(xb[bi][:, h * NQ:(h + 1) * NQ]),
            "wqt": wqt, "wkt": wkt, "wvot": wvot,
            "bq": bq, "bk": bk, "ones_bf": _ONES_BF,
        })

    nc = _get_compiled()
    res = run_bass_kernel_spmd(nc, in_maps, core_ids=list(range(N_CORES)))

    out = np.empty((B, HW, C), dtype=np.float32)
    for core in range(N_CORES):
        bi, h = core // 2, core % 2
        out[bi, h * NQ:(h + 1) * NQ, :] = res.results[core]["out"]
    out += bop  # bo' = Wo@bv + bo, exact because softmax rows sum to 1
    return out.reshape(B, C, 64, 64)


# revision 28
# speedup vs baseline: 1.3696x; 1.0238x over previous
"""Trainium2 Bass kernel for nn_Attention_57080115364834.

Reference computation (B=4, C=512, H=W=64, N=H*W=4096 tokens):
    t = x.reshape(b, c, n).swapaxes(1, 2)          # (b, n, c)
    q, k, v = t@Wq.T+bq, t@Wk.T+bk, t@Wv.T+bv
    attn = softmax(q @ k.T / sqrt(c))              # (b, n, n)
    out = (attn @ v) @ Wo.T + bo                   # (b, n, c)
    return out.reshape(b, c, h, w)                 # raw view, no permute

Sharding: 8 cores = 4 batches x 2 query-halves. Each core holds the full
x[b] (C x N == t.T, the natural Trainium layout) so it computes its
batch's full K^T (c,n) and VW (n,c) locally, plus Q^T for its half.

Host-side algebra folds both post-attention linear steps away:
  - softmax rows sum to 1  =>  v bias becomes output bias bo' = Wo@bv+bo,
    applied on the host after gathering (a per-channel constant add).
  - (attn@v)@Wo.T == attn@(t@(Wo@Wv).T), so with Wvo = Wo@Wv precomputed
    on host the VW projection directly produces final-channel values.

Per-core dataflow (matmuls bf16, f32 PSUM; normalization f32/f32r):
  kT[c,m]   = Wk @ tC + bk    VW[m,c] = tC.T @ WvoT     (phase 1)
  qT[c,n]   = Wq @ tCq + bq  per 512-token chunk (chunk 0 hoisted into
              phase 1 so the PSUM pool handover never idles the PE)
  ST[m,n]   = kT.T @ qT ; P = exp(ST/sqrt(c)) on ScalarE (no max-sub)
  acc      += P (DVE, f32r, for the rowsums)
  OT[n,c]  += P-chunk.T @ VW          (PSUM-accum over m-tiles; output is
              token-major, so rowsums live one-per-partition)
  otsb      = OT   (PSUM->SBUF split DVE/ACT; frees the 4 OT banks so the
              next chunk's attnv matmuls never wait)
  rs[tl]    = acc-chunk.T @ ones      (4 tiny N=1 matmuls -> [128,4])
  out[n,c]  = otsb / rs  via GPSIMD normalize_recip (per-partition denom)
              -> DMA, token-major, no transpose needed anywhere.

The old channel-major design needed reciprocal+broadcast of a [1,512]
row: a single-lane 3.3us DVE reciprocal plus a K=1 broadcast matmul sat
in the serial chain of every chunk and fully exposed at kernel end.
Token-major output turns that into per-partition work on the otherwise
idle GPSIMD engine.

Engine queues are strict FIFO, so chunk nb's normalization ops are
spread across chunk nb+1 (rowsums after qproj, normalize+DMA at mt==2)
so no FIFO ever backs up behind a dependency chain.

A short warm-up burst of zero matmuls runs during the initial DMA
lead-in so the PE_HAM clock gate is already 8/8 when real work arrives.
"""

import sys

for _p in ("/opt/trn_rl_repo", "/root/.axon_site/_ro/trn_rl_repo"):
    if _p not in sys.path:
        sys.path.append(_p)

import numpy as np
import ml_dtypes

import concourse.bacc as bacc
import concourse.mybir as mybir
import concourse.tile as tile
from concourse.bass_utils import run_bass_kernel_spmd

DT = mybir.dt.float32
BF = mybir.dt.bfloat16
F8 = mybir.dt.float8e4
AFT = mybir.ActivationFunctionType
DR = mybir.MatmulPerfMode.DoubleRow

B, C, HW = 4, 512, 4096          # batch, channels, tokens per batch
NQ = HW // 2                     # q tokens per core (2048)
CK = C // 128                    # contraction chunks (4)
MT = HW // 128                   # key/value tiles (32)
NB = NQ // 512                   # q-chunks per core (4)
SCALE = 1.0 / float(np.sqrt(C))
N_CORES = 8
N_WARM = 16                      # HAM warm-up matmuls
MT8 = 24                         # m-tiles 0..MT8-1 scored in fp8 DoubleRow
                                 # (keys are order-invariant under softmax;
                                 # fp8 on 3/4 of them measures ~1.4e-2 total
                                 # rel err vs the 2e-2 gate, and doubles the
                                 # score-matmul rate on those tiles)
J8 = MT8 // 4                    # 512-token kT slices stored fp8 (6)

_compiled = None
_ONES_BF = np.ones(128, dtype=ml_dtypes.bfloat16)


def _build():
    nc = bacc.Bacc("TRN2", target_bir_lowering=False)

    xt_e = nc.declare_dram_parameter("xt", [C, HW], BF, isOutput=False)
    xq_e = nc.declare_dram_parameter("xq", [C, NQ], BF, isOutput=False)
    wqt_e = nc.declare_dram_parameter("wqt", [C, C], BF, isOutput=False)
    wkt_e = nc.declare_dram_parameter("wkt", [C, C], BF, isOutput=False)
    wvot_e = nc.declare_dram_parameter("wvot", [C, C], BF, isOutput=False)
    bq_e = nc.declare_dram_parameter("bq", [C], DT, isOutput=False)
    bk_e = nc.declare_dram_parameter("bk", [C], DT, isOutput=False)
    ones_bf_e = nc.declare_dram_parameter("ones_bf", [128], BF, isOutput=False)
    out_e = nc.declare_dram_parameter("out", [NQ, C], DT, isOutput=True)

    with tile.TileContext(nc) as tc:
        # ---- HAM warm-up: zero matmuls with no DMA dependency keep the
        # PE busy through the initial DMA lead-in so the clock gate is at
        # 8/8 when real matmuls arrive. Pool closes -> PSUM bank reused.
        with (
            tc.tile_pool(name="warm", bufs=1) as warm_pool,
            tc.tile_pool(name="warmps", bufs=1, space="PSUM") as warm_ps,
        ):
            warm_sb = warm_pool.tile([128, 512], BF, tag="warm", name="warm_sb")
            nc.gpsimd.memset(warm_sb[:], 0.0)
            warm_ps_t = warm_ps.tile([128, 512], DT, tag="warmps", name="warm_ps")
            for i in range(N_WARM):
                nc.tensor.matmul(
                    warm_ps_t[:], warm_sb[:, 0:128], warm_sb[:],
                    start=(i == 0), stop=(i == N_WARM - 1),
                )

        with (
            tc.tile_pool(name="kt", bufs=1) as kt_pool,
            tc.tile_pool(name="vv", bufs=1) as vv_pool,
            tc.tile_pool(name="wq", bufs=1) as wq_pool,
            tc.tile_pool(name="consts", bufs=1) as c_pool,
            tc.tile_pool(name="xqp", bufs=2) as xq_pool,
            tc.tile_pool(name="qcp", bufs=2) as qc_pool,
        ):
            # ---- persistent tiles ----
            # kT split by score precision: m-tiles < MT8 live fp8 in the
            # DoubleRow-paired layout [128, 2(channel-group), m], the rest bf16
            kt8 = [kt_pool.tile([128, 2, J8 * 512], F8, tag=f"k8{g}", name=f"k8{g}") for g in range(2)]
            kt_bf = [kt_pool.tile([128, (MT - MT8) * 128], BF, tag=f"k{i}", name=f"k{i}") for i in range(CK)]
            vw_sb = [vv_pool.tile([128, C], BF, tag=f"v{i}", name=f"v{i}") for i in range(MT)]
            wq_sb = [wq_pool.tile([128, C], BF, tag=f"wq{i}", name=f"wq{i}") for i in range(CK)]

            bq_t = c_pool.tile([128, CK], DT, tag="bq", name="bq_t")
            bk_t = c_pool.tile([128, CK], DT, tag="bk", name="bk_t")
            ones_col_b = c_pool.tile([128, 1], BF, tag="onescb", name="ones_col_b")

            def load_xq(nb):
                xqs = [xq_pool.tile([128, 512], BF, tag=f"xq{ci}", name=f"xq{ci}") for ci in range(CK)]
                for ci in range(CK):
                    nc.gpsimd.dma_start(
                        xqs[ci][:], xq_e[ci * 128:(ci + 1) * 128, nb * 512:(nb + 1) * 512]
                    )
                return xqs

            def project_q(xqs, ps_pool, ps_tag, ps_bufs):
                qcs = []
                qc8 = [qc_pool.tile([128, 2, 512], F8, tag=f"qc8{g}", name=f"qc8{g}") for g in range(2)]
                for co in range(CK):
                    pq = ps_pool.tile([128, 512], DT, tag=ps_tag, name="pq", bufs=ps_bufs)
                    for ci in range(CK):
                        nc.tensor.matmul(
                            pq[:], wq_sb[ci][:, co * 128:(co + 1) * 128],
                            xqs[ci][:], start=(ci == 0), stop=(ci == CK - 1),
                        )
                    qc = qc_pool.tile([128, 512], BF, tag=f"qc{co}", name=f"qc{co}")
                    nc.scalar.activation(qc[:], pq[:], AFT.Identity, bias=bq_t[:, co:co + 1])
                    nc.scalar.activation(qc8[co // 2][:, co % 2, :], pq[:], AFT.Identity,
                                         bias=bq_t[:, co:co + 1])
                    qcs.append(qc)
                return qcs, qc8

            # ---- phase 1: kT (c,m) and VW (m,c) projections ----
            with (
                tc.tile_pool(name="wkv", bufs=1) as wkv_pool,
                tc.tile_pool(name="tcc", bufs=3) as tcc_pool,
                tc.tile_pool(name="ps1", bufs=2, space="PSUM") as ps1,
            ):
                wk_sb = [wkv_pool.tile([128, C], BF, tag=f"wk{i}", name=f"wk{i}") for i in range(CK)]
                wv_sb = [wkv_pool.tile([128, C], BF, tag=f"wv{i}", name=f"wv{i}") for i in range(CK)]

                # lead-in-critical DMAs first, in consumption order, alternating
                # trigger queues so two transfers are always in flight
                tcs01 = [tcc_pool.tile([128, 512], BF, tag=f"tc{ci}", name=f"tc{ci}") for ci in range(CK)]
                for ci in range(CK):
                    (nc.sync if ci % 2 == 0 else nc.gpsimd).dma_start(
                        tcs01[ci][:], xt_e[ci * 128:(ci + 1) * 128, 0:512])
                for i in range(CK):
                    (nc.gpsimd if i % 2 == 0 else nc.sync).dma_start(
                        wk_sb[i][:], wkt_e[i * 128:(i + 1) * 128, :])
                for i in range(CK):
                    (nc.sync if i % 2 == 0 else nc.gpsimd).dma_start(
                        wv_sb[i][:], wvot_e[i * 128:(i + 1) * 128, :])
                tcs1 = [tcc_pool.tile([128, 512], BF, tag=f"tc{ci}", name=f"tc{ci}") for ci in range(CK)]
                for ci in range(CK):
                    (nc.gpsimd if ci % 2 == 0 else nc.sync).dma_start(
                        tcs1[ci][:], xt_e[ci * 128:(ci + 1) * 128, 512:1024])
                for t in range(CK):
                    nc.sync.dma_start(bk_t[:, t:t + 1], bk_e[t * 128:(t + 1) * 128])
                nc.sync.dma_start(ones_col_b[:, 0:1], ones_bf_e[:])
                for i in range(CK):
                    nc.sync.dma_start(wq_sb[i][:], wqt_e[i * 128:(i + 1) * 128, :])
                for t in range(CK):
                    nc.sync.dma_start(bq_t[:, t:t + 1], bq_e[t * 128:(t + 1) * 128])

                qcs0 = None
                for j in range(HW // 512):
                    if j == 0:
                        tcs = tcs01
                    elif j == 1:
                        tcs = tcs1
                    else:
                        tcs = [tcc_pool.tile([128, 512], BF, tag=f"tc{ci}", name=f"tc{ci}") for ci in range(CK)]
                        for ci in range(CK):
                            nc.gpsimd.dma_start(
                                tcs[ci][:], xt_e[ci * 128:(ci + 1) * 128, j * 512:(j + 1) * 512]
                            )
                    # kT token-chunk j, all four output-channel chunks
                    for co in range(CK):
                        pk = ps1.tile([128, 512], DT, tag="pk", name="pk")
                        for ci in range(CK):
                            nc.tensor.matmul(
                                pk[:], wk_sb[ci][:, co * 128:(co + 1) * 128],
                                tcs[ci][:], start=(ci == 0), stop=(ci == CK - 1),
                            )
                        if j < J8:
                            dst = kt8[co // 2][:, co % 2, j * 512:(j + 1) * 512]
                        else:
                            dst = kt_bf[co][:, (j - J8) * 512:(j - J8 + 1) * 512]
                        nc.scalar.activation(dst, pk[:], AFT.Identity,
                                             bias=bk_t[:, co:co + 1])
                    # VW m-tiles 4j..4j+3 (no bias: folded into bo'),
                    # evacuation split DVE/ACT to balance engine backlogs.
                    for ml in range(4):
                        pv = ps1.tile([128, 512], DT, tag="pv", name="pv")
                        for ci in range(CK):
                            nc.tensor.matmul(
                                pv[:], tcs[ci][:, ml * 128:(ml + 1) * 128],
                                wv_sb[ci][:], start=(ci == 0), stop=(ci == CK - 1),
                            )
                        if ml % 2 == 0:
                            nc.vector.tensor_copy(vw_sb[4 * j + ml][:], pv[:])
                        else:
                            nc.scalar.activation(vw_sb[4 * j + ml][:], pv[:], AFT.Copy)
                    # chunk-0 q projection rides inside phase 1 so phase 2
                    # can open with score matmuls immediately
                    if j == 4:
                        qcs0, qc80 = project_q(load_xq(0), ps1, "pq", 1)

            # ---- phase 2: attention per 512-token q-chunk ----
            with (
                tc.tile_pool(name="pexp", bufs=8) as pe_pool,
                tc.tile_pool(name="accp", bufs=2) as acc_pool,
                tc.tile_pool(name="otsbp", bufs=2) as otsb_pool,
                tc.tile_pool(name="outp", bufs=3) as out_pool,
                tc.tile_pool(name="smallp", bufs=2) as small_pool,
                tc.tile_pool(name="ps2", bufs=4, space="PSUM") as ps2,
                tc.tile_pool(name="psot", bufs=1, space="PSUM") as psot,
            ):
                def tail_rowsums(acc_bf):
                    # transposed rowsums: [128 tokens, 1] per 128-token group,
                    # so the denominators land one-per-partition (fp32r can't
                    # be a wide stationary operand, hence the bf16 acc copy;
                    # the 128-way sum averages the rounding noise to ~4e-4)
                    rs_ps = ps2.tile([128, CK], DT, tag="st", name="rs_ps")
                    for tl in range(CK):
                        nc.tensor.matmul(
                            rs_ps[:, tl:tl + 1], acc_bf[:, tl * 128:(tl + 1) * 128],
                            ones_col_b[:, 0:1],
                            start=(tl == 0), stop=(tl == CK - 1),
                            skip_group_check=(tl > 0),
                        )
                    rs_sb = small_pool.tile([128, CK], DT, tag="rssb", name="rs_sb")
                    nc.vector.tensor_copy(rs_sb[:], rs_ps[:])
                    return rs_sb

                def tail_norm_out(tnb, tl, otsb_t, rs_sb):
                    fin = out_pool.tile([128, C], DT, tag="fin", name="fin", bufs=5)
                    nc.gpsimd.normalize_recip(fin[:], otsb_t[:], rs_sb[:, tl:tl + 1])
                    # out-DMA on the sync queue: a gpsimd dma_start's 650ns
                    # descriptor generation would serialize with the next
                    # normalize_recip on the gpsimd engine
                    nc.sync.dma_start(
                        out_e[tnb * 512 + tl * 128:tnb * 512 + (tl + 1) * 128, :], fin[:]
                    )

                prev = None
                xqs_next = load_xq(1)
                for nb in range(NB):
                    qcs, qc8 = (qcs0, qc80) if nb == 0 else project_q(xqs_next, ps2, "st", None)
                    if nb > 0 and nb + 1 < NB:
                        xqs_next = load_xq(nb + 1)
                    if prev is not None:
                        prev = (*prev, tail_rowsums(prev[1]))

                    # bf16 accumulator: 2x DVE rate, valid as a matmul
                    # stationary operand, and the rounding noise averages to
                    # ~4e-4 across the 128-way rowsum
                    acc = acc_pool.tile([128, 512], BF, tag="acc", name="acc")
                    ots = [psot.tile([128, C], DT, tag=f"ot{tl}", name=f"ot{tl}") for tl in range(CK)]
                    for mt in range(MT):
                        st = ps2.tile([128, 512], DT, tag="st", name="st")
                        if mt < MT8:
                            for g in range(2):
                                nc.tensor.matmul(
                                    st[:], kt8[g][:, :, mt * 128:(mt + 1) * 128],
                                    qc8[g][:, :, :], start=(g == 0), stop=(g == 1),
                                    perf_mode=DR,
                                )
                        else:
                            for ci in range(CK):
                                nc.tensor.matmul(
                                    st[:], kt_bf[ci][:, (mt - MT8) * 128:(mt - MT8 + 1) * 128],
                                    qcs[ci][:], start=(ci == 0), stop=(ci == CK - 1),
                                )
                        pexp = pe_pool.tile([128, 512], BF, tag="pe", name="pexp")
                        nc.scalar.activation(pexp[:], st[:], AFT.Exp, scale=SCALE)
                        if mt == 0:
                            nc.vector.tensor_copy(acc[:], pexp[:])
                        else:
                            nc.vector.tensor_add(acc[:], acc[:], pexp[:])
                        for tl in range(CK):
                            nc.tensor.matmul(
                                ots[tl][:], pexp[:, tl * 128:(tl + 1) * 128],
                                vw_sb[mt][:],
                                start=(mt == 0), stop=(mt == MT - 1), skip_group_check=True,
                            )
                        if mt == 2 and prev is not None:
                            tnb, _, otsb, rs_sb = prev
                            for tl in range(CK):
                                tail_norm_out(tnb, tl, otsb[tl], rs_sb)

                    # evacuate unnormalized accumulators (DVE/ACT split) ->
                    # frees the 4 OT PSUM banks for the next chunk at once
                    otsb = []
                    for tl in range(CK):
                        ob = otsb_pool.tile([128, C], DT, tag=f"ob{tl}", name=f"ob{tl}")
                        if tl % 2 == 0:
                            nc.vector.tensor_copy(ob[:], ots[tl][:])
                        else:
                            nc.scalar.activation(ob[:], ots[tl][:], AFT.Copy)
                        otsb.append(ob)
                    prev = (nb, acc, otsb)

                # last chunk: emit its whole tail immediately
                tnb, acc, otsb = prev
                rs_sb = tail_rowsums(acc)
                for tl in range(CK):
                    tail_norm_out(tnb, tl, otsb[tl], rs_sb)

    nc.compile()
    return nc


def _get_compiled():
    global _compiled
    if _compiled is None:
        _compiled = _build()
    return _compiled


def kernel(**inputs):
    x = np.ascontiguousarray(np.asarray(inputs["x"], dtype=np.float32))
    wq = np.asarray(inputs["Wq"], dtype=np.float32)
    wk = np.asarray(inputs["Wk"], dtype=np.float32)
    wv = np.asarray(inputs["Wv"], dtype=np.float32)
    wo = np.asarray(inputs["Wo"], dtype=np.float32)
    bq = np.ascontiguousarray(np.asarray(inputs["bq"], dtype=np.float32))
    bk = np.ascontiguousarray(np.asarray(inputs["bk"], dtype=np.float32))
    bv = np.asarray(inputs["bv"], dtype=np.float32)
    bo = np.asarray(inputs["bo"], dtype=np.float32)

    wqt = np.ascontiguousarray(wq.T.astype(ml_dtypes.bfloat16))
    wkt = np.ascontiguousarray(wk.T.astype(ml_dtypes.bfloat16))
    wvot = np.ascontiguousarray((wo @ wv).T.astype(ml_dtypes.bfloat16))
    bop = wo @ bv + bo

    xb = x.reshape(B, C, HW).astype(ml_dtypes.bfloat16)
    in_maps = []
    for core in range(N_CORES):
        bi, h = core // 2, core % 2
        in_maps.append({
            "xt": np.ascontiguousarray(xb[bi]),
            "xq": np.ascontiguousarray(xb[bi][:, h * NQ:(h + 1) * NQ]),
            "wqt": wqt, "wkt": wkt, "wvot": wvot,
            "bq": bq, "bk": bk, "ones_bf": _ONES_BF,
        })

    nc = _get_compiled()
    res = run_bass_kernel_spmd(nc, in_maps, core_ids=list(range(N_CORES)))

    out = np.empty((B, HW, C), dtype=np.float32)
    for core in range(N_CORES):
        bi, h = core // 2, core % 2
        out[bi, h * NQ:(h + 1) * NQ, :] = res.results[core]["out"]
    out += bop  # bo' = Wo@bv + bo, exact because softmax rows sum to 1
    return out.reshape(B, C, 64, 64)


# revision 29
# speedup vs baseline: 1.4105x; 1.0299x over previous
"""Trainium2 Bass kernel for nn_Attention_57080115364834.

Reference computation (B=4, C=512, H=W=64, N=H*W=4096 tokens):
    t = x.reshape(b, c, n).swapaxes(1, 2)          # (b, n, c)
    q, k, v = t@Wq.T+bq, t@Wk.T+bk, t@Wv.T+bv
    attn = softmax(q @ k.T / sqrt(c))              # (b, n, n)
    out = (attn @ v) @ Wo.T + bo                   # (b, n, c)
    return out.reshape(b, c, h, w)                 # raw view, no permute

Sharding: 8 cores = 4 batches x 2 query-halves. Each core holds the full
x[b] (C x N == t.T, the natural Trainium layout) so it computes its
batch's full K^T (c,n) and VW (n,c) locally, plus Q^T for its half.

Host-side algebra folds both post-attention linear steps away:
  - softmax rows sum to 1  =>  v bias becomes output bias bo' = Wo@bv+bo,
    applied on the host after gathering (a per-channel constant add).
  - (attn@v)@Wo.T == attn@(t@(Wo@Wv).T), so with Wvo = Wo@Wv precomputed
    on host the VW projection directly produces final-channel values.

Per-core dataflow (matmuls bf16, f32 PSUM; normalization f32/f32r):
  kT[c,m]   = Wk @ tC + bk    VW[m,c] = tC.T @ WvoT     (phase 1)
  qT[c,n]   = Wq @ tCq + bq  per 512-token chunk (chunk 0 hoisted into
              phase 1 so the PSUM pool handover never idles the PE)
  ST[m,n]   = kT.T @ qT ; P = exp(ST/sqrt(c)) on ScalarE (no max-sub)
  acc      += P (DVE, f32r, for the rowsums)
  OT[n,c]  += P-chunk.T @ VW          (PSUM-accum over m-tiles; output is
              token-major, so rowsums live one-per-partition)
  otsb      = OT   (PSUM->SBUF split DVE/ACT; frees the 4 OT banks so the
              next chunk's attnv matmuls never wait)
  rs[tl]    = acc-chunk.T @ ones      (4 tiny N=1 matmuls -> [128,4])
  out[n,c]  = otsb / rs  via GPSIMD normalize_recip (per-partition denom)
              -> DMA, token-major, no transpose needed anywhere.

The old channel-major design needed reciprocal+broadcast of a [1,512]
row: a single-lane 3.3us DVE reciprocal plus a K=1 broadcast matmul sat
in the serial chain of every chunk and fully exposed at kernel end.
Token-major output turns that into per-partition work on the otherwise
idle GPSIMD engine.

Engine queues are strict FIFO, so chunk nb's normalization ops are
spread across chunk nb+1 (rowsums after qproj, normalize+DMA at mt==2)
so no FIFO ever backs up behind a dependency chain.

A short warm-up burst of zero matmuls runs during the initial DMA
lead-in so the PE_HAM clock gate is already 8/8 when real work arrives.
"""

import sys

for _p in ("/opt/trn_rl_repo", "/root/.axon_site/_ro/trn_rl_repo"):
    if _p not in sys.path:
        sys.path.append(_p)

import numpy as np
import ml_dtypes

import concourse.bacc as bacc
import concourse.mybir as mybir
import concourse.tile as tile
from concourse.bass_utils import run_bass_kernel_spmd

DT = mybir.dt.float32
BF = mybir.dt.bfloat16
F8 = mybir.dt.float8e4
AFT = mybir.ActivationFunctionType
DR = mybir.MatmulPerfMode.DoubleRow

B, C, HW = 4, 512, 4096          # batch, channels, tokens per batch
NQ = HW // 2                     # q tokens per core (2048)
CK = C // 128                    # contraction chunks (4)
MT = HW // 128                   # key/value tiles (32)
NB = NQ // 512                   # q-chunks per core (4)
SCALE = 1.0 / float(np.sqrt(C))
N_CORES = 8
N_WARM = 10                      # HAM warm-up matmuls
MT8 = 28                         # m-tiles 0..MT8-1 scored in fp8 DoubleRow
                                 # (keys are order-invariant under softmax;
                                 # fp8 on 28/32 of them measures ~1.6e-2 total
                                 # rel err vs the 2e-2 gate, and doubles the
                                 # score-matmul rate on those tiles)
J8 = MT8 // 4                    # 512-token kT slices stored fp8 (6)

_compiled = None
_ONES_BF = np.ones(128, dtype=ml_dtypes.bfloat16)


def _build():
    nc = bacc.Bacc("TRN2", target_bir_lowering=False)

    xt_e = nc.declare_dram_parameter("xt", [C, HW], BF, isOutput=False)
    xq_e = nc.declare_dram_parameter("xq", [C, NQ], BF, isOutput=False)
    wqt_e = nc.declare_dram_parameter("wqt", [C, C], BF, isOutput=False)
    wkt_e = nc.declare_dram_parameter("wkt", [C, C], BF, isOutput=False)
    wvot_e = nc.declare_dram_parameter("wvot", [C, C], BF, isOutput=False)
    bq_e = nc.declare_dram_parameter("bq", [C], DT, isOutput=False)
    bk_e = nc.declare_dram_parameter("bk", [C], DT, isOutput=False)
    ones_bf_e = nc.declare_dram_parameter("ones_bf", [128], BF, isOutput=False)
    out_e = nc.declare_dram_parameter("out", [NQ, C], DT, isOutput=True)

    with tile.TileContext(nc) as tc:
        # ---- HAM warm-up: zero matmuls with no DMA dependency keep the
        # PE busy through the initial DMA lead-in so the clock gate is at
        # 8/8 when real matmuls arrive. Pool closes -> PSUM bank reused.
        with (
            tc.tile_pool(name="warm", bufs=1) as warm_pool,
            tc.tile_pool(name="warmps", bufs=1, space="PSUM") as warm_ps,
        ):
            warm_sb = warm_pool.tile([128, 512], BF, tag="warm", name="warm_sb")
            nc.gpsimd.memset(warm_sb[:], 0.0)
            warm_ps_t = warm_ps.tile([128, 512], DT, tag="warmps", name="warm_ps")
            for i in range(N_WARM):
                nc.tensor.matmul(
                    warm_ps_t[:], warm_sb[:, 0:128], warm_sb[:],
                    start=(i == 0), stop=(i == N_WARM - 1),
                )

        with (
            tc.tile_pool(name="kt", bufs=1) as kt_pool,
            tc.tile_pool(name="vv", bufs=1) as vv_pool,
            tc.tile_pool(name="wq", bufs=1) as wq_pool,
            tc.tile_pool(name="consts", bufs=1) as c_pool,
            tc.tile_pool(name="xqp", bufs=2) as xq_pool,
            tc.tile_pool(name="qcp", bufs=2) as qc_pool,
        ):
            # ---- persistent tiles ----
            # kT split by score precision: m-tiles < MT8 live fp8 in the
            # DoubleRow-paired layout [128, 2(channel-group), m], the rest bf16
            kt8 = [kt_pool.tile([128, 2, J8 * 512], F8, tag=f"k8{g}", name=f"k8{g}") for g in range(2)]
            kt_bf = [kt_pool.tile([128, (MT - MT8) * 128], BF, tag=f"k{i}", name=f"k{i}") for i in range(CK)]
            vw_sb = [vv_pool.tile([128, C], BF, tag=f"v{i}", name=f"v{i}") for i in range(MT)]
            wq_sb = [wq_pool.tile([128, C], BF, tag=f"wq{i}", name=f"wq{i}") for i in range(CK)]

            bq_t = c_pool.tile([128, CK], DT, tag="bq", name="bq_t")
            bk_t = c_pool.tile([128, CK], DT, tag="bk", name="bk_t")
            ones_col_b = c_pool.tile([128, 1], BF, tag="onescb", name="ones_col_b")

            def load_xq(nb):
                xqs = [xq_pool.tile([128, 512], BF, tag=f"xq{ci}", name=f"xq{ci}") for ci in range(CK)]
                for ci in range(CK):
                    nc.gpsimd.dma_start(
                        xqs[ci][:], xq_e[ci * 128:(ci + 1) * 128, nb * 512:(nb + 1) * 512]
                    )
                return xqs

            def project_q(xqs, ps_pool, ps_tag, ps_bufs):
                qcs = []
                qc8 = [qc_pool.tile([128, 2, 512], F8, tag=f"qc8{g}", name=f"qc8{g}") for g in range(2)]
                for co in range(CK):
                    pq = ps_pool.tile([128, 512], DT, tag=ps_tag, name="pq", bufs=ps_bufs)
                    for ci in range(CK):
                        nc.tensor.matmul(
                            pq[:], wq_sb[ci][:, co * 128:(co + 1) * 128],
                            xqs[ci][:], start=(ci == 0), stop=(ci == CK - 1),
                        )
                    qc = qc_pool.tile([128, 512], BF, tag=f"qc{co}", name=f"qc{co}")
                    nc.scalar.activation(qc[:], pq[:], AFT.Identity, bias=bq_t[:, co:co + 1])
                    nc.scalar.activation(qc8[co // 2][:, co % 2, :], pq[:], AFT.Identity,
                                         bias=bq_t[:, co:co + 1])
                    qcs.append(qc)
                return qcs, qc8

            # ---- phase 1: kT (c,m) and VW (m,c) projections ----
            with (
                tc.tile_pool(name="wkv", bufs=1) as wkv_pool,
                tc.tile_pool(name="tcc", bufs=3) as tcc_pool,
                tc.tile_pool(name="ps1", bufs=2, space="PSUM") as ps1,
            ):
                wk_sb = [wkv_pool.tile([128, C], BF, tag=f"wk{i}", name=f"wk{i}") for i in range(CK)]
                wv_sb = [wkv_pool.tile([128, C], BF, tag=f"wv{i}", name=f"wv{i}") for i in range(CK)]

                # all phase-1 inputs on the sync trigger queue (cheap
                # descriptors, unlike gpsimd's ~650ns DIRECT2D generation),
                # strictly in consumption order
                tcs_all = {}
                def load_tcs(j):
                    tcs = [tcc_pool.tile([128, 512], BF, tag=f"tc{ci}", name=f"tc{ci}") for ci in range(CK)]
                    for ci in range(CK):
                        nc.sync.dma_start(tcs[ci][:], xt_e[ci * 128:(ci + 1) * 128, j * 512:(j + 1) * 512])
                    tcs_all[j] = tcs
                load_tcs(0)
                for i in range(CK):
                    nc.sync.dma_start(wk_sb[i][:], wkt_e[i * 128:(i + 1) * 128, :])
                for i in range(CK):
                    nc.sync.dma_start(wv_sb[i][:], wvot_e[i * 128:(i + 1) * 128, :])
                load_tcs(1)
                load_tcs(2)
                for t in range(CK):
                    nc.sync.dma_start(bk_t[:, t:t + 1], bk_e[t * 128:(t + 1) * 128])
                nc.sync.dma_start(ones_col_b[:, 0:1], ones_bf_e[:])
                for i in range(CK):
                    nc.sync.dma_start(wq_sb[i][:], wqt_e[i * 128:(i + 1) * 128, :])
                for t in range(CK):
                    nc.sync.dma_start(bq_t[:, t:t + 1], bq_e[t * 128:(t + 1) * 128])

                qcs0 = None
                for j in range(HW // 512):
                    if j + 3 < HW // 512:
                        load_tcs(j + 3)
                    tcs = tcs_all.pop(j)
                    # kT token-chunk j, all four output-channel chunks
                    for co in range(CK):
                        pk = ps1.tile([128, 512], DT, tag="pk", name="pk")
                        for ci in range(CK):
                            nc.tensor.matmul(
                                pk[:], wk_sb[ci][:, co * 128:(co + 1) * 128],
                                tcs[ci][:], start=(ci == 0), stop=(ci == CK - 1),
                            )
                        if j < J8:
                            dst = kt8[co // 2][:, co % 2, j * 512:(j + 1) * 512]
                        else:
                            dst = kt_bf[co][:, (j - J8) * 512:(j - J8 + 1) * 512]
                        nc.scalar.activation(dst, pk[:], AFT.Identity,
                                             bias=bk_t[:, co:co + 1])
                    # VW m-tiles 4j..4j+3 (no bias: folded into bo'),
                    # evacuation split DVE/ACT to balance engine backlogs.
                    for ml in range(4):
                        pv = ps1.tile([128, 512], DT, tag="pv", name="pv")
                        for ci in range(CK):
                            nc.tensor.matmul(
                                pv[:], tcs[ci][:, ml * 128:(ml + 1) * 128],
                                wv_sb[ci][:], start=(ci == 0), stop=(ci == CK - 1),
                            )
                        if ml % 2 == 0:
                            nc.vector.tensor_copy(vw_sb[4 * j + ml][:], pv[:])
                        else:
                            nc.scalar.activation(vw_sb[4 * j + ml][:], pv[:], AFT.Copy)
                    # chunk-0 q projection rides inside phase 1 so phase 2
                    # can open with score matmuls immediately
                    if j == 4:
                        qcs0, qc80 = project_q(load_xq(0), ps1, "pq", 1)

            # ---- phase 2: attention per 512-token q-chunk ----
            with (
                tc.tile_pool(name="pexp", bufs=8) as pe_pool,
                tc.tile_pool(name="accp", bufs=2) as acc_pool,
                tc.tile_pool(name="otsbp", bufs=2) as otsb_pool,
                tc.tile_pool(name="outp", bufs=3) as out_pool,
                tc.tile_pool(name="smallp", bufs=2) as small_pool,
                tc.tile_pool(name="ps2", bufs=4, space="PSUM") as ps2,
                tc.tile_pool(name="psot", bufs=1, space="PSUM") as psot,
            ):
                def tail_rowsums(acc_bf):
                    # transposed rowsums: [128 tokens, 1] per 128-token group,
                    # so the denominators land one-per-partition (fp32r can't
                    # be a wide stationary operand, hence the bf16 acc copy;
                    # the 128-way sum averages the rounding noise to ~4e-4)
                    rs_ps = ps2.tile([128, CK], DT, tag="st", name="rs_ps")
                    for tl in range(CK):
                        nc.tensor.matmul(
                            rs_ps[:, tl:tl + 1], acc_bf[:, tl * 128:(tl + 1) * 128],
                            ones_col_b[:, 0:1],
                            start=(tl == 0), stop=(tl == CK - 1),
                            skip_group_check=(tl > 0),
                        )
                    rs_sb = small_pool.tile([128, CK], DT, tag="rssb", name="rs_sb")
                    nc.vector.tensor_copy(rs_sb[:], rs_ps[:])
                    return rs_sb

                def tail_norm_out(tnb, tl, otsb_t, rs_sb):
                    fin = out_pool.tile([128, C], DT, tag="fin", name="fin", bufs=5)
                    nc.gpsimd.normalize_recip(fin[:], otsb_t[:], rs_sb[:, tl:tl + 1])
                    # out-DMA on the sync queue: a gpsimd dma_start's 650ns
                    # descriptor generation would serialize with the next
                    # normalize_recip on the gpsimd engine
                    nc.sync.dma_start(
                        out_e[tnb * 512 + tl * 128:tnb * 512 + (tl + 1) * 128, :], fin[:]
                    )

                prev = None
                xqs_next = load_xq(1)
                for nb in range(NB):
                    qcs, qc8 = (qcs0, qc80) if nb == 0 else project_q(xqs_next, ps2, "st", None)
                    if nb > 0 and nb + 1 < NB:
                        xqs_next = load_xq(nb + 1)
                    if prev is not None:
                        prev = (*prev, tail_rowsums(prev[1]))

                    # bf16 accumulator: 2x DVE rate, valid as a matmul
                    # stationary operand, and the rounding noise averages to
                    # ~4e-4 across the 128-way rowsum
                    acc = acc_pool.tile([128, 512], BF, tag="acc", name="acc")
                    ots = [psot.tile([128, C], DT, tag=f"ot{tl}", name=f"ot{tl}") for tl in range(CK)]
                    for mt in range(MT):
                        st = ps2.tile([128, 512], DT, tag="st", name="st")
                        if mt < MT8:
                            for g in range(2):
                                nc.tensor.matmul(
                                    st[:], kt8[g][:, :, mt * 128:(mt + 1) * 128],
                                    qc8[g][:, :, :], start=(g == 0), stop=(g == 1),
                                    perf_mode=DR,
                                )
                        else:
                            for ci in range(CK):
                                nc.tensor.matmul(
                                    st[:], kt_bf[ci][:, (mt - MT8) * 128:(mt - MT8 + 1) * 128],
                                    qcs[ci][:], start=(ci == 0), stop=(ci == CK - 1),
                                )
                        pexp = pe_pool.tile([128, 512], BF, tag="pe", name="pexp")
                        nc.scalar.activation(pexp[:], st[:], AFT.Exp, scale=SCALE)
                        if mt == 0:
                            nc.vector.tensor_copy(acc[:], pexp[:])
                        else:
                            nc.vector.tensor_add(acc[:], acc[:], pexp[:])
                        for tl in range(CK):
                            nc.tensor.matmul(
                                ots[tl][:], pexp[:, tl * 128:(tl + 1) * 128],
                                vw_sb[mt][:],
                                start=(mt == 0), stop=(mt == MT - 1), skip_group_check=True,
                            )
                        if mt == 2 and prev is not None:
                            tnb, _, otsb, rs_sb = prev
                            for tl in range(CK):
                                tail_norm_out(tnb, tl, otsb[tl], rs_sb)

                    # evacuate unnormalized accumulators (DVE/ACT split) ->
                    # frees the 4 OT PSUM banks for the next chunk at once
                    otsb = []
                    for tl in range(CK):
                        ob = otsb_pool.tile([128, C], DT, tag=f"ob{tl}", name=f"ob{tl}")
                        if tl % 2 == 0:
                            nc.vector.tensor_copy(ob[:], ots[tl][:])
                        else:
                            nc.scalar.activation(ob[:], ots[tl][:], AFT.Copy)
                        otsb.append(ob)
                    prev = (nb, acc, otsb)

                # last chunk: emit its whole tail immediately
                tnb, acc, otsb = prev
                rs_sb = tail_rowsums(acc)
                for tl in range(CK):
                    tail_norm_out(tnb, tl, otsb[tl], rs_sb)

    nc.compile()
    return nc


def _get_compiled():
    global _compiled
    if _compiled is None:
        _compiled = _build()
    return _compiled


def kernel(**inputs):
    x = np.ascontiguousarray(np.asarray(inputs["x"], dtype=np.float32))
    wq = np.asarray(inputs["Wq"], dtype=np.float32)
    wk = np.asarray(inputs["Wk"], dtype=np.float32)
    wv = np.asarray(inputs["Wv"], dtype=np.float32)
    wo = np.asarray(inputs["Wo"], dtype=np.float32)
    bq = np.ascontiguousarray(np.asarray(inputs["bq"], dtype=np.float32))
    bk = np.ascontiguousarray(np.asarray(inputs["bk"], dtype=np.float32))
    bv = np.asarray(inputs["bv"], dtype=np.float32)
    bo = np.asarray(inputs["bo"], dtype=np.float32)

    wqt = np.ascontiguousarray(wq.T.astype(ml_dtypes.bfloat16))
    wkt = np.ascontiguousarray(wk.T.astype(ml_dtypes.bfloat16))
    wvot = np.ascontiguousarray((wo @ wv).T.astype(ml_dtypes.bfloat16))
    bop = wo @ bv + bo

    xb = x.reshape(B, C, HW).astype(ml_dtypes.bfloat16)
    in_maps = []
    for core in range(N_CORES):
        bi, h = core // 2, core % 2
        in_maps.append({
            "xt": np.ascontiguousarray(xb[bi]),
            "xq": np.ascontiguousarray(xb[bi][:, h * NQ:(h + 1) * NQ]),
            "wqt": wqt, "wkt": wkt, "wvot": wvot,
            "bq": bq, "bk": bk, "ones_bf": _ONES_BF,
        })

    nc = _get_compiled()
    res = run_bass_kernel_spmd(nc, in_maps, core_ids=list(range(N_CORES)))

    out = np.empty((B, HW, C), dtype=np.float32)
    for core in range(N_CORES):
        bi, h = core // 2, core % 2
        out[bi, h * NQ:(h + 1) * NQ, :] = res.results[core]["out"]
    out += bop  # bo' = Wo@bv + bo, exact because softmax rows sum to 1
    return out.reshape(B, C, 64, 64)


# revision 30
# speedup vs baseline: 1.4319x; 1.0152x over previous
"""Trainium2 Bass kernel for nn_Attention_57080115364834.

Reference computation (B=4, C=512, H=W=64, N=H*W=4096 tokens):
    t = x.reshape(b, c, n).swapaxes(1, 2)          # (b, n, c)
    q, k, v = t@Wq.T+bq, t@Wk.T+bk, t@Wv.T+bv
    attn = softmax(q @ k.T / sqrt(c))              # (b, n, n)
    out = (attn @ v) @ Wo.T + bo                   # (b, n, c)
    return out.reshape(b, c, h, w)                 # raw view, no permute

Sharding: 8 cores = 4 batches x 2 query-halves. Each core holds the full
x[b] (C x N == t.T, the natural Trainium layout) so it computes its
batch's full K^T (c,n) and VW (n,c) locally, plus Q^T for its half.

Host-side algebra folds both post-attention linear steps away:
  - softmax rows sum to 1  =>  v bias becomes output bias bo' = Wo@bv+bo,
    applied on the host after gathering (a per-channel constant add).
  - (attn@v)@Wo.T == attn@(t@(Wo@Wv).T), so with Wvo = Wo@Wv precomputed
    on host the VW projection directly produces final-channel values.

Per-core dataflow (matmuls bf16, f32 PSUM; normalization f32/f32r):
  kT[c,m]   = Wk @ tC + bk    VW[m,c] = tC.T @ WvoT     (phase 1)
  qT[c,n]   = Wq @ tCq + bq  per 512-token chunk (chunk 0 hoisted into
              phase 1 so the PSUM pool handover never idles the PE)
  ST[m,n]   = kT.T @ qT ; P = exp(ST/sqrt(c)) on ScalarE (no max-sub)
  acc      += P (DVE, f32r, for the rowsums)
  OT[n,c]  += P-chunk.T @ VW          (PSUM-accum over m-tiles; output is
              token-major, so rowsums live one-per-partition)
  otsb      = OT   (PSUM->SBUF split DVE/ACT; frees the 4 OT banks so the
              next chunk's attnv matmuls never wait)
  rs[tl]    = acc-chunk.T @ ones      (4 tiny N=1 matmuls -> [128,4])
  out[n,c]  = otsb / rs  via GPSIMD normalize_recip (per-partition denom)
              -> DMA, token-major, no transpose needed anywhere.

The old channel-major design needed reciprocal+broadcast of a [1,512]
row: a single-lane 3.3us DVE reciprocal plus a K=1 broadcast matmul sat
in the serial chain of every chunk and fully exposed at kernel end.
Token-major output turns that into per-partition work on the otherwise
idle GPSIMD engine.

Engine queues are strict FIFO, so chunk nb's normalization ops are
spread across chunk nb+1 (rowsums after qproj, normalize+DMA at mt==2)
so no FIFO ever backs up behind a dependency chain.

A short warm-up burst of zero matmuls runs during the initial DMA
lead-in so the PE_HAM clock gate is already 8/8 when real work arrives.
"""

import sys

for _p in ("/opt/trn_rl_repo", "/root/.axon_site/_ro/trn_rl_repo"):
    if _p not in sys.path:
        sys.path.append(_p)

import numpy as np
import ml_dtypes

import concourse.bacc as bacc
import concourse.mybir as mybir
import concourse.tile as tile
from concourse.bass_utils import run_bass_kernel_spmd

DT = mybir.dt.float32
BF = mybir.dt.bfloat16
F8 = mybir.dt.float8e4
AFT = mybir.ActivationFunctionType
DR = mybir.MatmulPerfMode.DoubleRow

B, C, HW = 4, 512, 4096          # batch, channels, tokens per batch
NQ = HW // 2                     # q tokens per core (2048)
CK = C // 128                    # contraction chunks (4)
MT = HW // 128                   # key/value tiles (32)
NB = NQ // 512                   # q-chunks per core (4)
SCALE = 1.0 / float(np.sqrt(C))
N_CORES = 8
N_WARM = 10                      # HAM warm-up matmuls
MT8 = 28                         # m-tiles 0..MT8-1 scored in fp8 DoubleRow
                                 # (keys are order-invariant under softmax;
                                 # fp8 on 28/32 of them measures ~1.6e-2 total
                                 # rel err vs the 2e-2 gate, and doubles the
                                 # score-matmul rate on those tiles)
J8 = MT8 // 4                    # 512-token kT slices stored fp8 (6)

_compiled = None
_ONES_BF = np.ones(128, dtype=ml_dtypes.bfloat16)


def _build():
    nc = bacc.Bacc("TRN2", target_bir_lowering=False)

    xt_e = nc.declare_dram_parameter("xt", [C, HW], BF, isOutput=False)
    xq_e = nc.declare_dram_parameter("xq", [C, NQ], BF, isOutput=False)
    wqt_e = nc.declare_dram_parameter("wqt", [C, C], BF, isOutput=False)
    wkt_e = nc.declare_dram_parameter("wkt", [C, C], BF, isOutput=False)
    wvot_e = nc.declare_dram_parameter("wvot", [C, C], BF, isOutput=False)
    bq_e = nc.declare_dram_parameter("bq", [C], DT, isOutput=False)
    bk_e = nc.declare_dram_parameter("bk", [C], DT, isOutput=False)
    ones_bf_e = nc.declare_dram_parameter("ones_bf", [128], BF, isOutput=False)
    out_e = nc.declare_dram_parameter("out", [NQ, C], DT, isOutput=True)

    with tile.TileContext(nc) as tc:
        # ---- HAM warm-up: zero matmuls with no DMA dependency keep the
        # PE busy through the initial DMA lead-in so the clock gate is at
        # 8/8 when real matmuls arrive. Pool closes -> PSUM bank reused.
        with (
            tc.tile_pool(name="warm", bufs=1) as warm_pool,
            tc.tile_pool(name="warmps", bufs=1, space="PSUM") as warm_ps,
        ):
            warm_sb = warm_pool.tile([128, 512], BF, tag="warm", name="warm_sb")
            nc.gpsimd.memset(warm_sb[:], 0.0)
            warm_ps_t = warm_ps.tile([128, 512], DT, tag="warmps", name="warm_ps")
            for i in range(N_WARM):
                nc.tensor.matmul(
                    warm_ps_t[:], warm_sb[:, 0:128], warm_sb[:],
                    start=(i == 0), stop=(i == N_WARM - 1),
                )

        with (
            tc.tile_pool(name="kt", bufs=1) as kt_pool,
            tc.tile_pool(name="vv", bufs=1) as vv_pool,
            tc.tile_pool(name="wq", bufs=1) as wq_pool,
            tc.tile_pool(name="consts", bufs=1) as c_pool,
            tc.tile_pool(name="xqp", bufs=2) as xq_pool,
            tc.tile_pool(name="qcp", bufs=2) as qc_pool,
        ):
            # ---- persistent tiles ----
            # kT split by score precision: m-tiles < MT8 live fp8 in the
            # DoubleRow-paired layout [128, 2(channel-group), m], the rest bf16
            kt8 = [kt_pool.tile([128, 2, J8 * 512], F8, tag=f"k8{g}", name=f"k8{g}") for g in range(2)]
            kt_bf = [kt_pool.tile([128, (MT - MT8) * 128], BF, tag=f"k{i}", name=f"k{i}") for i in range(CK)]
            vw_sb = [vv_pool.tile([128, C], BF, tag=f"v{i}", name=f"v{i}") for i in range(MT)]
            wq_sb = [wq_pool.tile([128, C], BF, tag=f"wq{i}", name=f"wq{i}") for i in range(CK)]

            bq_t = c_pool.tile([128, CK], DT, tag="bq", name="bq_t")
            bk_t = c_pool.tile([128, CK], DT, tag="bk", name="bk_t")
            ones_col_b = c_pool.tile([128, 1], BF, tag="onescb", name="ones_col_b")

            def load_xq(nb):
                xqs = [xq_pool.tile([128, 512], BF, tag=f"xq{ci}", name=f"xq{ci}") for ci in range(CK)]
                for ci in range(CK):
                    nc.gpsimd.dma_start(
                        xqs[ci][:], xq_e[ci * 128:(ci + 1) * 128, nb * 512:(nb + 1) * 512]
                    )
                return xqs

            def project_q(xqs, ps_pool, ps_tag, ps_bufs):
                qcs = []
                qc8 = [qc_pool.tile([128, 2, 512], F8, tag=f"qc8{g}", name=f"qc8{g}") for g in range(2)]
                for co in range(CK):
                    pq = ps_pool.tile([128, 512], DT, tag=ps_tag, name="pq", bufs=ps_bufs)
                    for ci in range(CK):
                        nc.tensor.matmul(
                            pq[:], wq_sb[ci][:, co * 128:(co + 1) * 128],
                            xqs[ci][:], start=(ci == 0), stop=(ci == CK - 1),
                        )
                    qc = qc_pool.tile([128, 512], BF, tag=f"qc{co}", name=f"qc{co}")
                    nc.scalar.activation(qc[:], pq[:], AFT.Identity, bias=bq_t[:, co:co + 1])
                    nc.scalar.activation(qc8[co // 2][:, co % 2, :], pq[:], AFT.Identity,
                                         bias=bq_t[:, co:co + 1])
                    qcs.append(qc)
                return qcs, qc8

            # ---- phase 1: kT (c,m) and VW (m,c) projections ----
            with (
                tc.tile_pool(name="wkv", bufs=1) as wkv_pool,
                tc.tile_pool(name="tcc", bufs=3) as tcc_pool,
                tc.tile_pool(name="ps1", bufs=2, space="PSUM") as ps1,
            ):
                wk_sb = [wkv_pool.tile([128, C], BF, tag=f"wk{i}", name=f"wk{i}") for i in range(CK)]
                wv_sb = [wkv_pool.tile([128, C], BF, tag=f"wv{i}", name=f"wv{i}") for i in range(CK)]

                # all phase-1 inputs on the sync trigger queue (cheap
                # descriptors, unlike gpsimd's ~650ns DIRECT2D generation),
                # strictly in consumption order
                tcs_all = {}
                def load_tcs(j):
                    tcs = [tcc_pool.tile([128, 512], BF, tag=f"tc{ci}", name=f"tc{ci}") for ci in range(CK)]
                    for ci in range(CK):
                        nc.sync.dma_start(tcs[ci][:], xt_e[ci * 128:(ci + 1) * 128, j * 512:(j + 1) * 512])
                    tcs_all[j] = tcs
                load_tcs(0)
                for i in range(CK):
                    nc.sync.dma_start(wk_sb[i][:], wkt_e[i * 128:(i + 1) * 128, :])
                for i in range(CK):
                    nc.sync.dma_start(wv_sb[i][:], wvot_e[i * 128:(i + 1) * 128, :])
                load_tcs(1)
                load_tcs(2)
                for t in range(CK):
                    nc.sync.dma_start(bk_t[:, t:t + 1], bk_e[t * 128:(t + 1) * 128])
                nc.sync.dma_start(ones_col_b[:, 0:1], ones_bf_e[:])
                for i in range(CK):
                    nc.sync.dma_start(wq_sb[i][:], wqt_e[i * 128:(i + 1) * 128, :])
                for t in range(CK):
                    nc.sync.dma_start(bq_t[:, t:t + 1], bq_e[t * 128:(t + 1) * 128])

                qcs0 = None
                for j in range(HW // 512):
                    if j + 3 < HW // 512:
                        load_tcs(j + 3)
                    tcs = tcs_all.pop(j)
                    # kT token-chunk j, all four output-channel chunks
                    for co in range(CK):
                        pk = ps1.tile([128, 512], DT, tag="pk", name="pk")
                        for ci in range(CK):
                            nc.tensor.matmul(
                                pk[:], wk_sb[ci][:, co * 128:(co + 1) * 128],
                                tcs[ci][:], start=(ci == 0), stop=(ci == CK - 1),
                            )
                        if j < J8:
                            dst = kt8[co // 2][:, co % 2, j * 512:(j + 1) * 512]
                        else:
                            dst = kt_bf[co][:, (j - J8) * 512:(j - J8 + 1) * 512]
                        nc.scalar.activation(dst, pk[:], AFT.Identity,
                                             bias=bk_t[:, co:co + 1])
                    # VW m-tiles 4j..4j+3 (no bias: folded into bo'),
                    # evacuation split DVE/ACT to balance engine backlogs.
                    for ml in range(4):
                        pv = ps1.tile([128, 512], DT, tag="pv", name="pv")
                        for ci in range(CK):
                            nc.tensor.matmul(
                                pv[:], tcs[ci][:, ml * 128:(ml + 1) * 128],
                                wv_sb[ci][:], start=(ci == 0), stop=(ci == CK - 1),
                            )
                        if ml % 2 == 0:
                            nc.vector.tensor_copy(vw_sb[4 * j + ml][:], pv[:])
                        else:
                            nc.scalar.activation(vw_sb[4 * j + ml][:], pv[:], AFT.Copy)
                    # chunk-0 q projection rides inside phase 1 so phase 2
                    # can open with score matmuls immediately
                    if j == 4:
                        qcs0, qc80 = project_q(load_xq(0), ps1, "pq", 1)

            # ---- phase 2: attention per 512-token q-chunk ----
            with (
                tc.tile_pool(name="pexp", bufs=8) as pe_pool,
                tc.tile_pool(name="accp", bufs=2) as acc_pool,
                tc.tile_pool(name="otsbp", bufs=2) as otsb_pool,
                tc.tile_pool(name="outp", bufs=3) as out_pool,
                tc.tile_pool(name="smallp", bufs=2) as small_pool,
                tc.tile_pool(name="ps2", bufs=4, space="PSUM") as ps2,
                tc.tile_pool(name="psot", bufs=1, space="PSUM") as psot,
            ):
                def tail_rowsums(acc_bf):
                    # transposed rowsums: [128 tokens, 1] per 128-token group,
                    # so the denominators land one-per-partition (fp32r can't
                    # be a wide stationary operand, hence the bf16 acc copy;
                    # the 128-way sum averages the rounding noise to ~4e-4)
                    rs_ps = ps2.tile([128, CK], DT, tag="st", name="rs_ps")
                    for tl in range(CK):
                        nc.tensor.matmul(
                            rs_ps[:, tl:tl + 1], acc_bf[:, tl * 128:(tl + 1) * 128],
                            ones_col_b[:, 0:1],
                            start=(tl == 0), stop=(tl == CK - 1),
                            skip_group_check=(tl > 0),
                        )
                    rs_sb = small_pool.tile([128, CK], DT, tag="rssb", name="rs_sb")
                    nc.vector.tensor_copy(rs_sb[:], rs_ps[:])
                    return rs_sb

                def tail_norm_out(tnb, tl, otsb_t, rs_sb):
                    fin = out_pool.tile([128, C], DT, tag="fin", name="fin", bufs=5)
                    nc.gpsimd.normalize_recip(fin[:], otsb_t[:], rs_sb[:, tl:tl + 1])
                    # out-DMA on the sync queue: a gpsimd dma_start's 650ns
                    # descriptor generation would serialize with the next
                    # normalize_recip on the gpsimd engine
                    nc.sync.dma_start(
                        out_e[tnb * 512 + tl * 128:tnb * 512 + (tl + 1) * 128, :], fin[:]
                    )

                prev = None
                xqs_next = load_xq(1)
                qcs_next = None
                for nb in range(NB):
                    qcs, qc8 = (qcs0, qc80) if nb == 0 else qcs_next
                    if nb > 0 and nb + 1 < NB:
                        xqs_next = load_xq(nb + 1)
                    if prev is not None:
                        prev = (*prev, tail_rowsums(prev[1]))

                    # bf16 accumulator: 2x DVE rate, valid as a matmul
                    # stationary operand, and the rounding noise averages to
                    # ~4e-4 across the 128-way rowsum
                    acc = acc_pool.tile([128, 512], BF, tag="acc", name="acc")
                    ots = [psot.tile([128, C], DT, tag=f"ot{tl}", name=f"ot{tl}") for tl in range(CK)]
                    for mt in range(MT):
                        st = ps2.tile([128, 512], DT, tag="st", name="st")
                        if mt < MT8:
                            for g in range(2):
                                nc.tensor.matmul(
                                    st[:], kt8[g][:, :, mt * 128:(mt + 1) * 128],
                                    qc8[g][:, :, :], start=(g == 0), stop=(g == 1),
                                    perf_mode=DR,
                                )
                        else:
                            for ci in range(CK):
                                nc.tensor.matmul(
                                    st[:], kt_bf[ci][:, (mt - MT8) * 128:(mt - MT8 + 1) * 128],
                                    qcs[ci][:], start=(ci == 0), stop=(ci == CK - 1),
                                )
                        pexp = pe_pool.tile([128, 512], BF, tag="pe", name="pexp")
                        nc.scalar.activation(pexp[:], st[:], AFT.Exp, scale=SCALE)
                        if mt == 0:
                            nc.vector.tensor_copy(acc[:], pexp[:])
                        else:
                            nc.vector.tensor_add(acc[:], acc[:], pexp[:])
                        for tl in range(CK):
                            nc.tensor.matmul(
                                ots[tl][:], pexp[:, tl * 128:(tl + 1) * 128],
                                vw_sb[mt][:],
                                start=(mt == 0), stop=(mt == MT - 1), skip_group_check=True,
                            )
                        if mt == 2 and prev is not None:
                            tnb, _, otsb, rs_sb = prev
                            for tl in range(CK):
                                tail_norm_out(tnb, tl, otsb[tl], rs_sb)
                        # next chunk's q-projection rides mid-loop where the
                        # PSUM rotation has slack, so chunk starts never stall
                        if mt == 26 and nb + 1 < NB:
                            qcs_next = project_q(xqs_next, ps2, "st", None)

                    # evacuate unnormalized accumulators (DVE/ACT split) ->
                    # frees the 4 OT PSUM banks for the next chunk at once
                    otsb = []
                    for tl in range(CK):
                        ob = otsb_pool.tile([128, C], DT, tag=f"ob{tl}", name=f"ob{tl}")
                        if tl % 2 == 0:
                            nc.vector.tensor_copy(ob[:], ots[tl][:])
                        else:
                            nc.scalar.activation(ob[:], ots[tl][:], AFT.Copy)
                        otsb.append(ob)
                    prev = (nb, acc, otsb)

                # last chunk: emit its whole tail immediately
                tnb, acc, otsb = prev
                rs_sb = tail_rowsums(acc)
                for tl in range(CK):
                    tail_norm_out(tnb, tl, otsb[tl], rs_sb)

    nc.compile()
    return nc


def _get_compiled():
    global _compiled
    if _compiled is None:
        _compiled = _build()
    return _compiled


def kernel(**inputs):
    x = np.ascontiguousarray(np.asarray(inputs["x"], dtype=np.float32))
    wq = np.asarray(inputs["Wq"], dtype=np.float32)
    wk = np.asarray(inputs["Wk"], dtype=np.float32)
    wv = np.asarray(inputs["Wv"], dtype=np.float32)
    wo = np.asarray(inputs["Wo"], dtype=np.float32)
    bq = np.ascontiguousarray(np.asarray(inputs["bq"], dtype=np.float32))
    bk = np.ascontiguousarray(np.asarray(inputs["bk"], dtype=np.float32))
    bv = np.asarray(inputs["bv"], dtype=np.float32)
    bo = np.asarray(inputs["bo"], dtype=np.float32)

    wqt = np.ascontiguousarray(wq.T.astype(ml_dtypes.bfloat16))
    wkt = np.ascontiguousarray(wk.T.astype(ml_dtypes.bfloat16))
    wvot = np.ascontiguousarray((wo @ wv).T.astype(ml_dtypes.bfloat16))
    bop = wo @ bv + bo

    xb = x.reshape(B, C, HW).astype(ml_dtypes.bfloat16)
    in_maps = []
    for core in range(N_CORES):
        bi, h = core // 2, core % 2
        in_maps.append({
            "xt": np.ascontiguousarray(xb[bi]),
            "xq": np.ascontiguousarray(xb[bi][:, h * NQ:(h + 1) * NQ]),
            "wqt": wqt, "wkt": wkt, "wvot": wvot,
            "bq": bq, "bk": bk, "ones_bf": _ONES_BF,
        })

    nc = _get_compiled()
    res = run_bass_kernel_spmd(nc, in_maps, core_ids=list(range(N_CORES)))

    out = np.empty((B, HW, C), dtype=np.float32)
    for core in range(N_CORES):
        bi, h = core // 2, core % 2
        out[bi, h * NQ:(h + 1) * NQ, :] = res.results[core]["out"]
    out += bop  # bo' = Wo@bv + bo, exact because softmax rows sum to 1
    return out.reshape(B, C, 64, 64)


# revision 31
# speedup vs baseline: 1.4410x; 1.0063x over previous
"""Trainium2 Bass kernel for nn_Attention_57080115364834.

Reference computation (B=4, C=512, H=W=64, N=H*W=4096 tokens):
    t = x.reshape(b, c, n).swapaxes(1, 2)          # (b, n, c)
    q, k, v = t@Wq.T+bq, t@Wk.T+bk, t@Wv.T+bv
    attn = softmax(q @ k.T / sqrt(c))              # (b, n, n)
    out = (attn @ v) @ Wo.T + bo                   # (b, n, c)
    return out.reshape(b, c, h, w)                 # raw view, no permute

Sharding: 8 cores = 4 batches x 2 query-halves. Each core holds the full
x[b] (C x N == t.T, the natural Trainium layout) so it computes its
batch's full K^T (c,n) and VW (n,c) locally, plus Q^T for its half.

Host-side algebra folds both post-attention linear steps away:
  - softmax rows sum to 1  =>  v bias becomes output bias bo' = Wo@bv+bo,
    applied on the host after gathering (a per-channel constant add).
  - (attn@v)@Wo.T == attn@(t@(Wo@Wv).T), so with Wvo = Wo@Wv precomputed
    on host the VW projection directly produces final-channel values.

Per-core dataflow (f32 PSUM accumulation everywhere):
  kT[c,m]   = Wk @ tC + bk    VW[m,c] = tC.T @ WvoT     (phase 1, bf16)
  qT[c,n]   = Wq @ tCq + bq  per 512-token chunk, evacuated twice: bf16
              and fp8e4 in the DoubleRow-paired [128, 2, n] layout
              (chunk 0 rides inside phase 1; later chunks are projected
              mid-m-loop of the previous chunk, where the PSUM rotation
              has slack, so chunk starts never stall the PE)
  ST[m,n]   = kT.T @ qT: m-tiles 0..MT8-1 as fp8 DoubleRow matmuls (2x
              rate, 2 MMs per tile), the rest bf16 (4 MMs). Key order is
              softmax-invariant, so the precision split is free.
  P = exp(ST/sqrt(c)) on ScalarE (no max-subtract: |scores/sqrt(c)|<~2)
  acc      += P (DVE, bf16: 2x rate; rounding averages out in rowsums)
  OT[n,c]  += P-chunk.T @ VW          (PSUM-accum over m-tiles; output is
              token-major, so rowsums live one-per-partition)
  otsb      = OT   (PSUM->SBUF split DVE/ACT; frees the 4 OT banks so the
              next chunk's attnv matmuls never wait)
  rs[tl]    = acc-chunk.T @ ones      (4 tiny N=1 matmuls -> [128,4])
  out[n,c]  = otsb / rs  via GPSIMD normalize_recip (per-partition denom)
              -> DMA, token-major, no transpose needed anywhere.

A channel-major output would need reciprocal+broadcast of a [1,512]
row: a single-lane 3.3us DVE reciprocal plus a K=1 broadcast matmul in
the serial chain of every chunk, fully exposed at kernel end.
Token-major output turns that into per-partition work on the otherwise
idle GPSIMD engine.

Engine queues are strict FIFO, so chunk nb's normalization ops are
spread across chunk nb+1 (rowsums after the chunk start, normalize+DMA
at mt==2, output DMAs on the sync queue) so no FIFO ever backs up
behind a dependency chain. Phase-1 inputs stream on the sync queue in
exact consumption order (gpsimd DMA descriptors cost ~650ns each and
would pace the start). A short warm-up burst of zero matmuls runs
during the initial DMA lead-in so the PE_HAM clock gate is already 8/8
when real work arrives.

Measured on 8 axon trn2 cores: 274.6us, rel err 1.59e-2 (gate 2e-2);
the v1 baseline was 357.9us at 3.7e-3.
"""

import sys

for _p in ("/opt/trn_rl_repo", "/root/.axon_site/_ro/trn_rl_repo"):
    if _p not in sys.path:
        sys.path.append(_p)

import numpy as np
import ml_dtypes

import concourse.bacc as bacc
import concourse.mybir as mybir
import concourse.tile as tile
from concourse.bass_utils import run_bass_kernel_spmd

DT = mybir.dt.float32
BF = mybir.dt.bfloat16
F8 = mybir.dt.float8e4
AFT = mybir.ActivationFunctionType
DR = mybir.MatmulPerfMode.DoubleRow

B, C, HW = 4, 512, 4096          # batch, channels, tokens per batch
NQ = HW // 2                     # q tokens per core (2048)
CK = C // 128                    # contraction chunks (4)
MT = HW // 128                   # key/value tiles (32)
NB = NQ // 512                   # q-chunks per core (4)
SCALE = 1.0 / float(np.sqrt(C))
N_CORES = 8
N_WARM = 10                      # HAM warm-up matmuls
MT8 = 28                         # m-tiles 0..MT8-1 scored in fp8 DoubleRow
                                 # (keys are order-invariant under softmax;
                                 # fp8 on 28/32 of them measures ~1.6e-2 total
                                 # rel err vs the 2e-2 gate, and doubles the
                                 # score-matmul rate on those tiles)
J8 = MT8 // 4                    # 512-token kT slices stored fp8 (6)

_compiled = None
_ONES_BF = np.ones(128, dtype=ml_dtypes.bfloat16)


def _build():
    nc = bacc.Bacc("TRN2", target_bir_lowering=False)

    xt_e = nc.declare_dram_parameter("xt", [C, HW], BF, isOutput=False)
    xq_e = nc.declare_dram_parameter("xq", [C, NQ], BF, isOutput=False)
    wqt_e = nc.declare_dram_parameter("wqt", [C, C], BF, isOutput=False)
    wkt_e = nc.declare_dram_parameter("wkt", [C, C], BF, isOutput=False)
    wvot_e = nc.declare_dram_parameter("wvot", [C, C], BF, isOutput=False)
    bq_e = nc.declare_dram_parameter("bq", [C], DT, isOutput=False)
    bk_e = nc.declare_dram_parameter("bk", [C], DT, isOutput=False)
    ones_bf_e = nc.declare_dram_parameter("ones_bf", [128], BF, isOutput=False)
    out_e = nc.declare_dram_parameter("out", [NQ, C], DT, isOutput=True)

    with tile.TileContext(nc) as tc:
        # ---- HAM warm-up: zero matmuls with no DMA dependency keep the
        # PE busy through the initial DMA lead-in so the clock gate is at
        # 8/8 when real matmuls arrive. Pool closes -> PSUM bank reused.
        with (
            tc.tile_pool(name="warm", bufs=1) as warm_pool,
            tc.tile_pool(name="warmps", bufs=1, space="PSUM") as warm_ps,
        ):
            warm_sb = warm_pool.tile([128, 512], BF, tag="warm", name="warm_sb")
            nc.gpsimd.memset(warm_sb[:], 0.0)
            warm_ps_t = warm_ps.tile([128, 512], DT, tag="warmps", name="warm_ps")
            for i in range(N_WARM):
                nc.tensor.matmul(
                    warm_ps_t[:], warm_sb[:, 0:128], warm_sb[:],
                    start=(i == 0), stop=(i == N_WARM - 1),
                )

        with (
            tc.tile_pool(name="kt", bufs=1) as kt_pool,
            tc.tile_pool(name="vv", bufs=1) as vv_pool,
            tc.tile_pool(name="wq", bufs=1) as wq_pool,
            tc.tile_pool(name="consts", bufs=1) as c_pool,
            tc.tile_pool(name="xqp", bufs=2) as xq_pool,
            tc.tile_pool(name="qcp", bufs=2) as qc_pool,
        ):
            # ---- persistent tiles ----
            # kT split by score precision: m-tiles < MT8 live fp8 in the
            # DoubleRow-paired layout [128, 2(channel-group), m], the rest bf16
            kt8 = [kt_pool.tile([128, 2, J8 * 512], F8, tag=f"k8{g}", name=f"k8{g}") for g in range(2)]
            kt_bf = [kt_pool.tile([128, (MT - MT8) * 128], BF, tag=f"k{i}", name=f"k{i}") for i in range(CK)]
            vw_sb = [vv_pool.tile([128, C], BF, tag=f"v{i}", name=f"v{i}") for i in range(MT)]
            wq_sb = [wq_pool.tile([128, C], BF, tag=f"wq{i}", name=f"wq{i}") for i in range(CK)]

            bq_t = c_pool.tile([128, CK], DT, tag="bq", name="bq_t")
            bk_t = c_pool.tile([128, CK], DT, tag="bk", name="bk_t")
            ones_col_b = c_pool.tile([128, 1], BF, tag="onescb", name="ones_col_b")

            def load_xq(nb):
                xqs = [xq_pool.tile([128, 512], BF, tag=f"xq{ci}", name=f"xq{ci}") for ci in range(CK)]
                for ci in range(CK):
                    nc.gpsimd.dma_start(
                        xqs[ci][:], xq_e[ci * 128:(ci + 1) * 128, nb * 512:(nb + 1) * 512]
                    )
                return xqs

            def project_q(xqs, ps_pool, ps_tag, ps_bufs):
                qcs = []
                qc8 = [qc_pool.tile([128, 2, 512], F8, tag=f"qc8{g}", name=f"qc8{g}") for g in range(2)]
                for co in range(CK):
                    pq = ps_pool.tile([128, 512], DT, tag=ps_tag, name="pq", bufs=ps_bufs)
                    for ci in range(CK):
                        nc.tensor.matmul(
                            pq[:], wq_sb[ci][:, co * 128:(co + 1) * 128],
                            xqs[ci][:], start=(ci == 0), stop=(ci == CK - 1),
                        )
                    qc = qc_pool.tile([128, 512], BF, tag=f"qc{co}", name=f"qc{co}")
                    nc.scalar.activation(qc[:], pq[:], AFT.Identity, bias=bq_t[:, co:co + 1])
                    nc.scalar.activation(qc8[co // 2][:, co % 2, :], pq[:], AFT.Identity,
                                         bias=bq_t[:, co:co + 1])
                    qcs.append(qc)
                return qcs, qc8

            # ---- phase 1: kT (c,m) and VW (m,c) projections ----
            with (
                tc.tile_pool(name="wkv", bufs=1) as wkv_pool,
                tc.tile_pool(name="tcc", bufs=3) as tcc_pool,
                tc.tile_pool(name="ps1", bufs=2, space="PSUM") as ps1,
            ):
                wk_sb = [wkv_pool.tile([128, C], BF, tag=f"wk{i}", name=f"wk{i}") for i in range(CK)]
                wv_sb = [wkv_pool.tile([128, C], BF, tag=f"wv{i}", name=f"wv{i}") for i in range(CK)]

                # all phase-1 inputs on the sync trigger queue (cheap
                # descriptors, unlike gpsimd's ~650ns DIRECT2D generation),
                # strictly in consumption order
                tcs_all = {}
                def load_tcs(j):
                    tcs = [tcc_pool.tile([128, 512], BF, tag=f"tc{ci}", name=f"tc{ci}") for ci in range(CK)]
                    for ci in range(CK):
                        nc.sync.dma_start(tcs[ci][:], xt_e[ci * 128:(ci + 1) * 128, j * 512:(j + 1) * 512])
                    tcs_all[j] = tcs
                load_tcs(0)
                for i in range(CK):
                    nc.sync.dma_start(wk_sb[i][:], wkt_e[i * 128:(i + 1) * 128, :])
                for i in range(CK):
                    nc.sync.dma_start(wv_sb[i][:], wvot_e[i * 128:(i + 1) * 128, :])
                load_tcs(1)
                load_tcs(2)
                for t in range(CK):
                    nc.sync.dma_start(bk_t[:, t:t + 1], bk_e[t * 128:(t + 1) * 128])
                nc.sync.dma_start(ones_col_b[:, 0:1], ones_bf_e[:])
                for i in range(CK):
                    nc.sync.dma_start(wq_sb[i][:], wqt_e[i * 128:(i + 1) * 128, :])
                for t in range(CK):
                    nc.sync.dma_start(bq_t[:, t:t + 1], bq_e[t * 128:(t + 1) * 128])

                qcs0 = None
                for j in range(HW // 512):
                    if j + 3 < HW // 512:
                        load_tcs(j + 3)
                    tcs = tcs_all.pop(j)
                    # kT token-chunk j, all four output-channel chunks
                    for co in range(CK):
                        pk = ps1.tile([128, 512], DT, tag="pk", name="pk")
                        for ci in range(CK):
                            nc.tensor.matmul(
                                pk[:], wk_sb[ci][:, co * 128:(co + 1) * 128],
                                tcs[ci][:], start=(ci == 0), stop=(ci == CK - 1),
                            )
                        if j < J8:
                            dst = kt8[co // 2][:, co % 2, j * 512:(j + 1) * 512]
                        else:
                            dst = kt_bf[co][:, (j - J8) * 512:(j - J8 + 1) * 512]
                        nc.scalar.activation(dst, pk[:], AFT.Identity,
                                             bias=bk_t[:, co:co + 1])
                    # VW m-tiles 4j..4j+3 (no bias: folded into bo'),
                    # evacuation split DVE/ACT to balance engine backlogs.
                    for ml in range(4):
                        pv = ps1.tile([128, 512], DT, tag="pv", name="pv")
                        for ci in range(CK):
                            nc.tensor.matmul(
                                pv[:], tcs[ci][:, ml * 128:(ml + 1) * 128],
                                wv_sb[ci][:], start=(ci == 0), stop=(ci == CK - 1),
                            )
                        if ml % 2 == 0:
                            nc.vector.tensor_copy(vw_sb[4 * j + ml][:], pv[:])
                        else:
                            nc.scalar.activation(vw_sb[4 * j + ml][:], pv[:], AFT.Copy)
                    # chunk-0 q projection rides inside phase 1 so phase 2
                    # can open with score matmuls immediately
                    if j == 4:
                        qcs0, qc80 = project_q(load_xq(0), ps1, "pq", 1)

            # ---- phase 2: attention per 512-token q-chunk ----
            with (
                tc.tile_pool(name="pexp", bufs=8) as pe_pool,
                tc.tile_pool(name="accp", bufs=2) as acc_pool,
                tc.tile_pool(name="otsbp", bufs=2) as otsb_pool,
                tc.tile_pool(name="outp", bufs=3) as out_pool,
                tc.tile_pool(name="smallp", bufs=2) as small_pool,
                tc.tile_pool(name="ps2", bufs=4, space="PSUM") as ps2,
                tc.tile_pool(name="psot", bufs=1, space="PSUM") as psot,
            ):
                def tail_rowsums(acc_bf):
                    # transposed rowsums: [128 tokens, 1] per 128-token group,
                    # so the denominators land one-per-partition (fp32r can't
                    # be a wide stationary operand, hence the bf16 acc copy;
                    # the 128-way sum averages the rounding noise to ~4e-4)
                    rs_ps = ps2.tile([128, CK], DT, tag="st", name="rs_ps")
                    for tl in range(CK):
                        nc.tensor.matmul(
                            rs_ps[:, tl:tl + 1], acc_bf[:, tl * 128:(tl + 1) * 128],
                            ones_col_b[:, 0:1],
                            start=(tl == 0), stop=(tl == CK - 1),
                            skip_group_check=(tl > 0),
                        )
                    rs_sb = small_pool.tile([128, CK], DT, tag="rssb", name="rs_sb")
                    nc.vector.tensor_copy(rs_sb[:], rs_ps[:])
                    return rs_sb

                def tail_norm_out(tnb, tl, otsb_t, rs_sb):
                    fin = out_pool.tile([128, C], DT, tag="fin", name="fin", bufs=5)
                    nc.gpsimd.normalize_recip(fin[:], otsb_t[:], rs_sb[:, tl:tl + 1])
                    # out-DMA on the sync queue: a gpsimd dma_start's 650ns
                    # descriptor generation would serialize with the next
                    # normalize_recip on the gpsimd engine
                    nc.sync.dma_start(
                        out_e[tnb * 512 + tl * 128:tnb * 512 + (tl + 1) * 128, :], fin[:]
                    )

                prev = None
                xqs_next = load_xq(1)
                qcs_next = None
                for nb in range(NB):
                    qcs, qc8 = (qcs0, qc80) if nb == 0 else qcs_next
                    if nb > 0 and nb + 1 < NB:
                        xqs_next = load_xq(nb + 1)
                    if prev is not None:
                        prev = (*prev, tail_rowsums(prev[1]))

                    # bf16 accumulator: 2x DVE rate, valid as a matmul
                    # stationary operand, and the rounding noise averages to
                    # ~4e-4 across the 128-way rowsum
                    acc = acc_pool.tile([128, 512], BF, tag="acc", name="acc")
                    ots = [psot.tile([128, C], DT, tag=f"ot{tl}", name=f"ot{tl}") for tl in range(CK)]
                    for mt in range(MT):
                        st = ps2.tile([128, 512], DT, tag="st", name="st")
                        if mt < MT8:
                            for g in range(2):
                                nc.tensor.matmul(
                                    st[:], kt8[g][:, :, mt * 128:(mt + 1) * 128],
                                    qc8[g][:, :, :], start=(g == 0), stop=(g == 1),
                                    perf_mode=DR,
                                )
                        else:
                            for ci in range(CK):
                                nc.tensor.matmul(
                                    st[:], kt_bf[ci][:, (mt - MT8) * 128:(mt - MT8 + 1) * 128],
                                    qcs[ci][:], start=(ci == 0), stop=(ci == CK - 1),
                                )
                        pexp = pe_pool.tile([128, 512], BF, tag="pe", name="pexp")
                        nc.scalar.activation(pexp[:], st[:], AFT.Exp, scale=SCALE)
                        if mt == 0:
                            nc.vector.tensor_copy(acc[:], pexp[:])
                        else:
                            nc.vector.tensor_add(acc[:], acc[:], pexp[:])
                        for tl in range(CK):
                            nc.tensor.matmul(
                                ots[tl][:], pexp[:, tl * 128:(tl + 1) * 128],
                                vw_sb[mt][:],
                                start=(mt == 0), stop=(mt == MT - 1), skip_group_check=True,
                            )
                        if mt == 2 and prev is not None:
                            tnb, _, otsb, rs_sb = prev
                            for tl in range(CK):
                                tail_norm_out(tnb, tl, otsb[tl], rs_sb)
                        # next chunk's q-projection rides mid-loop where the
                        # PSUM rotation has slack, so chunk starts never stall
                        if mt == 26 and nb + 1 < NB:
                            qcs_next = project_q(xqs_next, ps2, "st", None)

                    # evacuate unnormalized accumulators (DVE/ACT split) ->
                    # frees the 4 OT PSUM banks for the next chunk at once
                    otsb = []
                    for tl in range(CK):
                        ob = otsb_pool.tile([128, C], DT, tag=f"ob{tl}", name=f"ob{tl}")
                        if tl % 2 == 0:
                            nc.vector.tensor_copy(ob[:], ots[tl][:])
                        else:
                            nc.scalar.activation(ob[:], ots[tl][:], AFT.Copy)
                        otsb.append(ob)
                    prev = (nb, acc, otsb)

                # last chunk: emit its whole tail immediately
                tnb, acc, otsb = prev
                rs_sb = tail_rowsums(acc)
                for tl in range(CK):
                    tail_norm_out(tnb, tl, otsb[tl], rs_sb)

    nc.compile()
    return nc


def _get_compiled():
    global _compiled
    if _compiled is None:
        _compiled = _build()
    return _compiled


def kernel(**inputs):
    x = np.ascontiguousarray(np.asarray(inputs["x"], dtype=np.float32))
    wq = np.asarray(inputs["Wq"], dtype=np.float32)
    wk = np.asarray(inputs["Wk"], dtype=np.float32)
    wv = np.asarray(inputs["Wv"], dtype=np.float32)
    wo = np.asarray(inputs["Wo"], dtype=np.float32)
    bq = np.ascontiguousarray(np.asarray(inputs["bq"], dtype=np.float32))
    bk = np.ascontiguousarray(np.asarray(inputs["bk"], dtype=np.float32))
    bv = np.asarray(inputs["bv"], dtype=np.float32)
    bo = np.asarray(inputs["bo"], dtype=np.float32)

    wqt = np.ascontiguousarray(wq.T.astype(ml_dtypes.bfloat16))
    wkt = np.ascontiguousarray(wk.T.astype(ml_dtypes.bfloat16))
    wvot = np.ascontiguousarray((wo @ wv).T.astype(ml_dtypes.bfloat16))
    bop = wo @ bv + bo

    xb = x.reshape(B, C, HW).astype(ml_dtypes.bfloat16)
    in_maps = []
    for core in range(N_CORES):
        bi, h = core // 2, core % 2
        in_maps.append({
            "xt": np.ascontiguousarray(xb[bi]),
            "xq": np.ascontiguousarray(xb[bi][:, h * NQ:(h + 1) * NQ]),
            "wqt": wqt, "wkt": wkt, "wvot": wvot,
            "bq": bq, "bk": bk, "ones_bf": _ONES_BF,
        })

    nc = _get_compiled()
    res = run_bass_kernel_spmd(nc, in_maps, core_ids=list(range(N_CORES)))

    out = np.empty((B, HW, C), dtype=np.float32)
    for core in range(N_CORES):
        bi, h = core // 2, core % 2
        out[bi, h * NQ:(h + 1) * NQ, :] = res.results[core]["out"]
    out += bop  # bo' = Wo@bv + bo, exact because softmax rows sum to 1
    return out.reshape(B, C, 64, 64)
